# revision 8
# baseline (speedup 1.0000x reference)
"""AttentionGuidedInterpolation kernel for 8 Trainium2 NeuronCores.

Device (Bass/Tile, SPMD x8): the compute-heavy similarity search —
64 gram matrices (128-dim features, 1024x1024 each, 17.2 GFLOP) on the
TensorEngine, PSUM->SBUF bf16 downcast on the Activation engine, and
top-8 row search via the DVE Max8/MaxIndex instructions. Each core
handles 8 of the 64 independent (slice, batch) units.

The per-core schedule is DVE-roofline-bound: CoreSim shows the DVE at
94.6% occupancy with zero gaps (Max + MaxIndex are mandatory full-row
scans at 1 elem/lane/cycle; neither supports the 2x packed perf modes,
there is no fused max-with-index instruction, and no other engine can
pre-reduce a row for it). PE sits at 18%, Act at 52% — the algorithmic
floor for exact per-row top-k on this hardware.

Dispatch: run_bass_kernel_spmd under axon re-builds and re-jits its
shard_map wrapper on every call (new closure -> jit cache miss -> full
XLA retrace + compile + neuronx cache lookup, several hundred ms of
pure host/RPC overhead per call). We inline the same bass2jax lowering
it uses (_bass_exec_p custom call on the 8-device mesh) but cache the
jitted executable, pre-stage inputs asynchronously, and create the
donated output buffers on-device, so the per-call cost is one blocking
execute round-trip. Falls back to run_bass_kernel_spmd, then to a
numpy top-k, if the internal API is unavailable.

Host (numpy): index-weighted neighbor combine, grid samples, and the
tiny 4-token attention — cheap glue driven by the device-computed
indices, overlapped with the device call.
"""

import sys
import threading
import time

if "/opt/trn_rl_repo" not in sys.path:
    sys.path.insert(0, "/opt/trn_rl_repo")

import numpy as np

TOP_K = 5
R = 1
NUM_HEADS = 8
N, C, D, H, W, K = 4, 128, 16, 32, 32, 8192
S, L = D, H * W  # 16 slices, 1024 positions per slice
N_CORES = 8
UPC = (S * N) // N_CORES  # units per core = 8

_cache = {}


ENC_M = float(3 * 2**22)  # magic round-to-integer constant (ulp = 1)


def _build_bass(reps=1, algo="enc"):
    """Build the gram + top-k program.

    algo="enc" (default): single-DVE-scan encoded top-8. The Act engine
    rewrites each PSUM gram bank in place as fl(16*sim + M) — the f32
    add against M = 3*2^22 (ulp 1) rounds the similarity to an integer
    grid (quantum 1/16, finer than the bf16 ranking it replaces). Two
    cheap 1-row accumulate matmuls then add -M (exact by Sterbenz) and
    (1023-j)/1024 (exact: |Q| <= 2^13 leaves 10 mantissa bits for the
    fraction; every k/1024 is exact in fp16). One DVE max8 over the
    encoded PSUM returns the top-8 values with their column indices
    embedded in the fraction — no max_index scan. This halves the DVE
    work that bounds the 2-scan variant.

    algo="scan": the classic exact path (bf16 downcast + DVE Max +
    MaxIndex) — the runtime fallback if the encoded path's PSUM
    accumulate semantics differ on silicon (detected by the decode
    sanity check in _run_device_topk).

    reps > 1 unrolls the identical per-core workload `reps` times (only
    rep 0 stores outputs) — used by test.py to measure steady-state
    per-execution HW time differentially, cancelling the axon dispatch
    round-trip that dwarfs a single execution.
    """
    import concourse.mybir as mybir
    from concourse import bacc, tile
    from concourse._compat import get_trn_type

    f32 = mybir.dt.float32
    bf16 = mybir.dt.bfloat16
    f16 = mybir.dt.float16
    u16 = mybir.dt.uint16

    nc = bacc.Bacc(
        get_trn_type(),
        target_bir_lowering=False,
        debug=False,
        num_devices=N_CORES,
    )
    sl_in = nc.dram_tensor("sl", [UPC, 128, L], bf16, kind="ExternalInput")

    if algo == "enc":
        negM_in = nc.dram_tensor("negM", [1, L], bf16, kind="ExternalInput")
        frac_in = nc.dram_tensor("frac", [1, L], f16, kind="ExternalInput")
        enc_out = nc.dram_tensor("enc", [UPC, L, TOP_K], f32, kind="ExternalOutput")
        with tile.TileContext(nc) as tc:
            with (
                tc.tile_pool(name="sb", bufs=3) as pool,
                tc.tile_pool(name="cstp", bufs=6) as cstpool,
                tc.tile_pool(name="mxp", bufs=8) as mxpool,
                tc.tile_pool(name="ps", bufs=4, space="PSUM") as pp,
            ):
                negM = cstpool.tile([1, L], bf16, tag="negM")
                nc.sync.dma_start(out=negM[:], in_=negM_in[:])
                frac = cstpool.tile([1, L], f16, tag="frac")
                nc.sync.dma_start(out=frac[:], in_=frac_in[:])
                ones_bf = cstpool.tile([1, 128], bf16, tag="onesb")
                nc.vector.memset(ones_bf[:], 1.0)
                ones_f16 = cstpool.tile([1, 128], f16, tag="onesh")
                nc.vector.memset(ones_f16[:], 1.0)
                biasM = cstpool.tile([128, 1], f32, tag="biasM")
                nc.vector.memset(biasM[:], ENC_M)
                for rep in range(reps):
                    for u in range(UPC):
                        sl_t = pool.tile([128, L], bf16, tag="sl")
                        nc.sync.dma_start(out=sl_t[:], in_=sl_in[u])
                        for lt in range(L // 128):
                            ps = pp.tile([128, L], f32, tag="ps")
                            lhsT = sl_t[:, lt * 128 : (lt + 1) * 128]
                            for h in (0, 1):
                                sl_h = sl_t[:, h * 512 : (h + 1) * 512]
                                p = ps[:, h * 512 : (h + 1) * 512]
                                nc.tensor.matmul(p, lhsT, sl_h)
                                nc.scalar.activation(
                                    p,
                                    p,
                                    mybir.ActivationFunctionType.Identity,
                                    bias=biasM[:],
                                    scale=16.0,
                                )
                                nc.tensor.matmul(
                                    p,
                                    ones_bf[:],
                                    negM[:, h * 512 : (h + 1) * 512],
                                    start=False,
                                    stop=False,
                                    skip_group_check=True,
                                )
                                nc.tensor.matmul(
                                    p,
                                    ones_f16[:],
                                    frac[:, h * 512 : (h + 1) * 512],
                                    start=False,
                                    stop=True,
                                    skip_group_check=True,
                                )
                            mx = mxpool.tile([128, 8], f32, tag="mx")
                            nc.vector.max(mx[:], ps[:])
                            if rep == 0:
                                nc.sync.dma_start(
                                    out=enc_out[u, lt * 128 : (lt + 1) * 128, :],
                                    in_=mx[:, 0:TOP_K],
                                )
        nc.compile()
        return nc

    idxs_out = nc.dram_tensor("idxs", [UPC, L, TOP_K], u16, kind="ExternalOutput")
    with tile.TileContext(nc) as tc:
        with (
            tc.tile_pool(name="sb", bufs=3) as pool,
            tc.tile_pool(name="mxp", bufs=8) as mxpool,
            tc.tile_pool(name="simp", bufs=4) as simpool,
            tc.tile_pool(name="ps", bufs=3, space="PSUM") as pp,
        ):
            for rep in range(reps):
                for u in range(UPC):
                    sl_t = pool.tile([128, L], bf16, tag="sl")
                    nc.sync.dma_start(out=sl_t[:], in_=sl_in[u])
                    for lt in range(L // 128):
                        ps = pp.tile([128, L], f32, tag="ps")
                        lhsT = sl_t[:, lt * 128 : (lt + 1) * 128]
                        # two matmuls: a PSUM bank holds 512 fp32 per partition
                        nc.tensor.matmul(ps[:, 0:512], lhsT, sl_t[:, 0:512])
                        nc.tensor.matmul(ps[:, 512:1024], lhsT, sl_t[:, 512:1024])
                        # bf16 ranking copy: sim values only rank neighbors;
                        # the result is insensitive to rank jitter beyond the
                        # (always exact) self-match because its 1/1e-5 weight
                        # dominates the index-distance softmax.
                        sim_bf = simpool.tile([128, L], bf16, tag="sim")
                        nc.scalar.copy(sim_bf[:, 0:512], ps[:, 0:512])
                        nc.scalar.copy(sim_bf[:, 512:1024], ps[:, 512:1024])
                        mx = mxpool.tile([128, 8], bf16, tag="mx")
                        ix = mxpool.tile([128, 8], u16, tag="ix")
                        nc.vector.max(mx[:], sim_bf[:])
                        nc.vector.max_index(ix[:], mx[:], sim_bf[:])
                        if rep == 0:
                            nc.sync.dma_start(
                                out=idxs_out[u, lt * 128 : (lt + 1) * 128, :],
                                in_=ix[:, 0:TOP_K],
                            )
    nc.compile()
    return nc


class _CachedRunner:
    """bass2jax axon dispatch with a cached jitted executable.

    Mirrors what run_bass_kernel_spmd does under axon (the _bass_exec_p
    custom call inside a shard_map over the 8-core mesh) but builds the
    jit exactly once, so repeat calls skip the retrace/compile.
    """

    def __init__(self, nc):
        import jax
        import jax.numpy as jnp
        from jax.sharding import Mesh, NamedSharding, PartitionSpec

        try:
            from jax.experimental.shard_map import shard_map
        except ImportError:  # newer jax
            from jax import shard_map

        import concourse.mybir as mybir
        from concourse.bass2jax import (
            _bass_exec_p,
            install_neuronx_cc_hook,
            partition_id_tensor,
        )

        install_neuronx_cc_hook()
        self.jax = jax
        self.nc = nc

        partition_name = (
            nc.partition_id_tensor.name if nc.partition_id_tensor else None
        )
        in_names, out_names, out_avals, zero_shapes = [], [], [], []
        for alloc in nc.m.functions[0].allocations:
            if not isinstance(alloc, mybir.MemoryLocationSet):
                continue
            name = alloc.memorylocations[0].name
            if alloc.kind == "ExternalInput":
                if name != partition_name:
                    in_names.append(name)
            elif alloc.kind == "ExternalOutput":
                shape = tuple(alloc.tensor_shape)
                dtype = mybir.dt.np(alloc.dtype)
                out_names.append(name)
                out_avals.append(jax.core.ShapedArray(shape, dtype))
                zero_shapes.append((shape, dtype))
        n_params, n_outs = len(in_names), len(out_avals)
        self.in_names = in_names
        self.out_names = out_names
        in_names_all = in_names + out_names
        if partition_name is not None:
            in_names_all.append(partition_name)

        def _body(*args):
            ops = list(args)
            if partition_name is not None:
                ops.append(partition_id_tensor())
            return tuple(
                _bass_exec_p.bind(
                    *ops,
                    out_avals=tuple(out_avals),
                    in_names=tuple(in_names_all),
                    out_names=tuple(out_names),
                    lowering_input_output_aliases=(),
                    sim_require_finite=True,
                    sim_require_nnan=True,
                    nc=nc,
                )
            )

        devices = jax.devices()[:N_CORES]
        assert len(devices) == N_CORES
        mesh = Mesh(np.asarray(devices), ("core",))
        self.spec = NamedSharding(mesh, PartitionSpec("core"))
        self.sharded = jax.jit(
            shard_map(
                _body,
                mesh=mesh,
                in_specs=(PartitionSpec("core"),) * (n_params + n_outs),
                out_specs=(PartitionSpec("core"),) * n_outs,
                check_rep=False,
            ),
            donate_argnums=tuple(range(n_params, n_params + n_outs)),
            keep_unused=True,
        )
        # Donated output buffers, created on-device (no H2D bytes).
        self.mkzeros = jax.jit(
            lambda: tuple(
                jnp.zeros((N_CORES * s[0], *s[1:]), d) for s, d in zero_shapes
            ),
            out_shardings=(self.spec,) * n_outs,
        )

    def stage(self, in_map):
        """Async H2D of global (cores-concatenated) inputs + donated outputs.

        in_map: {name: global array with axis 0 = n_cores * per_core_dim0}.
        """
        dins = [self.jax.device_put(in_map[n], self.spec) for n in self.in_names]
        zs = self.mkzeros()
        return dins, zs

    def execute(self, dins, zs, block=True):
        """Dispatch the NEFF; returns (device outputs, blocking span ns)."""
        t0 = time.perf_counter()
        out = self.sharded(*dins, *zs)
        if block:
            self.jax.block_until_ready(out)
        span = (time.perf_counter() - t0) * 1e9
        return out, span

    def fetch(self, out):
        """D2H with all shard transfers in flight before any blocks."""
        arrs = []
        for o in out:
            shards = o.addressable_shards
            for s_ in shards:
                s_.data.copy_to_host_async()
            arrs.append(
                np.concatenate([np.asarray(s_.data) for s_ in shards], axis=0)
            )
        return dict(zip(self.out_names, arrs))


def _get_runner(reps=1, algo="enc"):
    key = ("runner", reps, algo)
    if key not in _cache:
        _cache[key] = _CachedRunner(_build_bass(reps=reps, algo=algo))
    return _cache[key]


def _enc_consts():
    """Per-core const rows for the encoded path, replicated across cores."""
    import ml_dtypes

    negM = np.full((N_CORES, L), -ENC_M, np.float32).astype(ml_dtypes.bfloat16)
    frac = np.tile(
        ((1023.0 - np.arange(L)) / 1024.0).astype(np.float16)[None, :], (N_CORES, 1)
    )
    return negM, frac


def _decode_enc(enc):
    """enc (64, L, 5) f32 -> idx int64 + sanity flag.

    enc = Q + (1023-j)/1024 with Q = round(16*sim) an integer: exact in
    f32 for |Q| < 2^13, so the decode recovers j exactly. Sanity: the
    top-1 of a gram row is its own diagonal (self-similarity dominates
    by ~8 quantization sigmas); if the PSUM round-trip behaved
    differently on silicon the fractions collapse and this check fails.
    """
    Q = np.floor(enc)
    j = 1023 - np.rint((enc - Q) * 1024.0).astype(np.int64)
    ok = bool((j >= 0).all() and (j <= 1023).all())
    if ok:
        diag = np.arange(L)[None, :]
        ok = float(np.mean(j[:, :, 0] == diag)) > 0.999
    return j, ok


def _host_topk(sl_full):
    """Numpy fallback: exact gram + top-5 (jax tie-break: value desc, index asc)."""
    slb = sl_full.reshape(S * N, C, L)
    sim = np.matmul(np.transpose(slb, (0, 2, 1)), slb).reshape(S, N, L, L)
    part = np.argpartition(-sim, TOP_K, axis=-1)[..., :TOP_K]
    pvals = np.take_along_axis(sim, part, axis=-1)
    order = np.lexsort((part, -pvals), axis=-1)
    idx = np.take_along_axis(part, order, axis=-1)
    return idx.astype(np.int64)  # (S,N,L,5)


def _to_bf16_units(sl_full):
    import ml_dtypes

    return np.ascontiguousarray(sl_full.reshape(S * N, C, L)).astype(
        ml_dtypes.bfloat16
    )


def _run_device_topk(sl_full):
    """sl_full: (S, N, C, L) f32. Returns idx (S,N,L,5) int64 via 8 cores."""
    sl_units = _to_bf16_units(sl_full)
    t0 = time.perf_counter()
    idx = None
    if not _cache.get("enc_bad"):
        try:  # encoded single-scan path
            runner = _get_runner(reps=1, algo="enc")
            negM, frac = _enc_consts()
            dins, zs = runner.stage({"sl": sl_units, "negM": negM, "frac": frac})
            out, span = runner.execute(dins, zs)
            res = runner.fetch(out)
            _cache["exec_span_ns"] = span
            j, ok = _decode_enc(res["enc"].reshape(S * N, L, TOP_K))
            if ok:
                idx = j
            else:  # silicon disagreed with the PSUM encode round-trip
                _cache["enc_bad"] = True
        except Exception:  # pragma: no cover - harness-proofing
            _cache["enc_bad"] = True
    if idx is None:
        try:  # exact 2-scan DVE path
            runner = _get_runner(reps=1, algo="scan")
            dins, zs = runner.stage({"sl": sl_units})
            out, span = runner.execute(dins, zs)
            res = runner.fetch(out)
            _cache["exec_span_ns"] = span
            idx = res["idxs"]
        except Exception:  # pragma: no cover
            from concourse.bass_utils import run_bass_kernel_spmd

            if "nc" not in _cache:
                _cache["nc"] = _build_bass(reps=1, algo="scan")
            in_maps = [
                {"sl": np.ascontiguousarray(sl_units[c * UPC : (c + 1) * UPC])}
                for c in range(N_CORES)
            ]
            out = run_bass_kernel_spmd(_cache["nc"], in_maps, list(range(N_CORES)))
            idx = np.concatenate(
                [np.asarray(out.results[c]["idxs"]) for c in range(N_CORES)], 0
            )
    _cache["last_device_ns"] = (time.perf_counter() - t0) * 1e9
    idx = np.clip(idx.reshape(S, N, L, TOP_K).astype(np.int64), 0, L - 1)
    return idx


# ---------------- numpy ports of the reference glue ----------------


def _unnorm(g, size):
    return ((g + 1.0) * size - 1.0) / 2.0


def _grid_sample_3d(fm, grid, mode, fmt=None):
    # fm: (N,C,Dd,Hh,Ww); grid: (N,P,3) last dim (x->W, y->H, z->D)
    # fmt: optional precomputed voxel-major view (N, D*H*W, C)
    n_, c_, d_, h_, w_ = fm.shape
    if fmt is None:
        fmt = np.ascontiguousarray(
            np.transpose(fm, (0, 2, 3, 4, 1)).reshape(n_, d_ * h_ * w_, c_)
        )
    ix = _unnorm(grid[..., 0], w_)
    iy = _unnorm(grid[..., 1], h_)
    iz = _unnorm(grid[..., 2], d_)

    def fetch(z, y, x):
        valid = (z >= 0) & (z < d_) & (y >= 0) & (y < h_) & (x >= 0) & (x < w_)
        lin = (
            np.clip(z, 0, d_ - 1) * (h_ * w_)
            + np.clip(y, 0, h_ - 1) * w_
            + np.clip(x, 0, w_ - 1)
        )
        v = np.take_along_axis(fmt, lin[..., None], axis=1)
        v[~valid] = 0.0
        return v

    if mode == "nearest":
        return fetch(
            np.round(iz).astype(np.int64),
            np.round(iy).astype(np.int64),
            np.round(ix).astype(np.int64),
        )
    x0 = np.floor(ix)
    y0 = np.floor(iy)
    z0 = np.floor(iz)
    tx, ty, tz = ix - x0, iy - y0, iz - z0
    x0i, y0i, z0i = x0.astype(np.int64), y0.astype(np.int64), z0.astype(np.int64)
    out = np.zeros(grid.shape[:-1] + (c_,), fm.dtype)
    for dz in (0, 1):
        for dy in (0, 1):
            for dx in (0, 1):
                wgt = (
                    (tz if dz else 1.0 - tz)
                    * (ty if dy else 1.0 - ty)
                    * (tx if dx else 1.0 - tx)
                ).astype(np.float32)
                out += fetch(z0i + dz, y0i + dy, x0i + dx) * wgt[..., None]
    return out  # (N,P,C)


def _nearest_lin(grid, d_, h_, w_):
    """Shared nearest-voxel linear indices + validity for a (N,P,3) grid."""
    ix = _unnorm(grid[..., 0], w_)
    iy = _unnorm(grid[..., 1], h_)
    iz = _unnorm(grid[..., 2], d_)
    z = np.round(iz).astype(np.int64)
    y = np.round(iy).astype(np.int64)
    x = np.round(ix).astype(np.int64)
    valid = (z >= 0) & (z < d_) & (y >= 0) & (y < h_) & (x >= 0) & (x < w_)
    lin = (
        np.clip(z, 0, d_ - 1) * (h_ * w_)
        + np.clip(y, 0, h_ - 1) * w_
        + np.clip(x, 0, w_ - 1)
    )
    return lin, valid


def _fetch_lin(fmt, lin, valid):
    v = np.take_along_axis(fmt, lin[..., None], axis=1)
    v[~valid] = 0.0
    return v


def _find_neighbor_coords(xyz_hr, fm_shape, r=R):
    d_, h_, w_ = fm_shape[-3:]
    scale = np.array([d_ - 1, h_ - 1, w_ - 1], np.float32)
    g = np.floor((xyz_hr + 1.0) / 2.0 * scale).astype(np.float32)
    steps = np.linspace(-float(r), float(r), 2 * r + 1).astype(np.float32)
    dh, dv = steps * np.float32(2.0 / h_), steps * np.float32(2.0 / w_)
    # mdi == 0 for these shapes (D=16 smallest)
    d2 = np.stack(np.meshgrid(dh, dv, indexing="ij"), -1).reshape(1, 1, -1, 2)
    nc2 = g[..., 1:][:, :, None, :] + d2
    fixed = np.broadcast_to(g[..., 0:1][:, :, None, :], nc2.shape[:3] + (1,))
    ncrd = np.concatenate([fixed, nc2], -1).astype(np.float32)
    return ncrd / scale * 2.0 - 1.0  # (N,K,A,3)


def kernel(**inputs):
    fm = np.asarray(inputs["feature_map"], np.float32)
    xyz = np.asarray(inputs["xyz_hr"], np.float32)
    Wq = np.asarray(inputs["Wq"], np.float32)
    bq = np.asarray(inputs["bq"], np.float32)
    Wk = np.asarray(inputs["Wk"], np.float32)
    bk = np.asarray(inputs["bk"], np.float32)
    Wv = np.asarray(inputs["Wv"], np.float32)
    bv = np.asarray(inputs["bv"], np.float32)
    ipw = np.asarray(inputs["in_proj_w"], np.float32)
    ipb = np.asarray(inputs["in_proj_b"], np.float32)
    ow = np.asarray(inputs["out_w"], np.float32)
    ob = np.asarray(inputs["out_b"], np.float32)

    # ---- similarity search: gram + top-8 on the 8 NeuronCores ----
    sl_full = np.ascontiguousarray(
        np.transpose(fm, (2, 0, 1, 3, 4)).reshape(S, N, C, L)
    )

    # Run the device top-k concurrently with the host-side sampling work
    # that does not depend on it (bilinear init_fv, neighbor coords, nf).
    dev = {}

    def _dev_worker():
        try:
            dev["idx"] = _run_device_topk(sl_full)  # (S,N,L,5)
        except Exception as e:  # device path unavailable -> host fallback
            dev["err"] = e

    th = threading.Thread(target=_dev_worker)
    th.start()

    # ---- device-independent sampling work (overlapped with the device call) ----
    fmt_fm = np.ascontiguousarray(
        np.transpose(fm, (0, 2, 3, 4, 1)).reshape(N, D * H * W, C)
    )
    init_fv = _grid_sample_3d(fm, xyz[..., ::-1], "bilinear", fmt=fmt_fm)  # (N,K,C)
    ncrd = _find_neighbor_coords(xyz, fm.shape)  # (N,K,A,3)
    A = ncrd.shape[2]
    grid_n = ncrd.reshape(N, K * A, 3)[..., ::-1]
    lin_n, valid_n = _nearest_lin(grid_n, D, H, W)  # shared by nf and sf
    nf = _fetch_lin(fmt_fm, lin_n, valid_n)
    rd = np.linalg.norm(
        xyz[:, :, None, None, :] - ncrd[:, :, None, :, :], axis=-1
    ).astype(np.float32)
    rw = 1.0 / (rd + np.float32(1e-6))
    rw = (rw / rw.sum(-1, keepdims=True)).reshape(N, K, 1, A)  # (N,K,1,A)

    th.join()
    idx = dev.get("idx")
    if idx is None:
        idx = _host_topk(sl_full)

    # ---- index-weighted neighbor combine (host) ----
    featsT = np.ascontiguousarray(np.transpose(sl_full, (0, 1, 3, 2))).reshape(
        S * N, L, C
    )
    dist = np.abs(idx - np.arange(L)[None, None, :, None]).astype(
        np.float32
    ) + np.float32(1e-5)
    w = 1.0 / dist
    w = (w / w.sum(-1, keepdims=True)).astype(np.float32).reshape(S * N, L, TOP_K)
    idx_f = idx.reshape(S * N, L, TOP_K)
    g5 = np.take_along_axis(
        featsT, idx_f.reshape(S * N, L * TOP_K, 1), axis=1
    ).reshape(S * N, L, TOP_K, C)
    wa_lc = (w[:, :, None, :] @ g5).reshape(S * N, L, C)
    # Direct permutation of wa_lc (S,N,L,C) to the voxel-major layout the
    # nearest-sample needs — equivalent to building sim_feats=(N,C,D,H,W) and
    # re-transposing, but with one copy instead of two. Index algebra:
    # sim_feats[n,c,d,h,w] = wa[4n + c//32, (c%32)//8, (c%8)*16 + d, h*32+w].
    sim_fmt = np.ascontiguousarray(
        wa_lc.reshape(4, 4, 4, L, 8, 16).transpose(0, 5, 3, 1, 2, 4)
    ).reshape(N, D * H * W, C)

    sf = _fetch_lin(sim_fmt, lin_n, valid_n)
    # comb = ((nf_v*rw).sum(2)+(sf_v*rw).sum(2))/2 == ((nf_v+sf_v)*rw).sum(2)/2,
    # so add before the raw (N,C,P)->(N,K,A,C) view and weight once. The
    # torch view maps (k,a,c) -> s[n, (k%64)*A*C + a*C + c, k//64] (since
    # P = K*A = 64*A*C here), so contract from that strided view directly
    # instead of materialising the 300MB (N,C,P) transpose copy.
    sr = (nf + sf).reshape(N, 64, A, C, 128)  # [n, k%64, a, c, k//64]
    rwr = rw.reshape(N, 128, 64, A)  # [n, k//64, k%64, a]
    comb = np.einsum("nqma,nmafq->nqmf", rwr, sr, optimize=True).reshape(
        N, K, C
    ) / np.float32(2.0)

    # ---- projections + 4-token attention (seq axis = N, batch = K) ----
    q = init_fv @ Wq.T + bq
    k = comb @ Wk.T + bk
    v = comb @ Wv.T + bv
    E = C
    hd = E // NUM_HEADS
    qp = (q @ ipw[:E].T + ipb[:E]).reshape(N, K, NUM_HEADS, hd)
    kp = (k @ ipw[E : 2 * E].T + ipb[E : 2 * E]).reshape(N, K, NUM_HEADS, hd)
    vp = (v @ ipw[2 * E :].T + ipb[2 * E :]).reshape(N, K, NUM_HEADS, hd)
    qb = np.ascontiguousarray(np.transpose(qp, (1, 2, 0, 3)))  # (K,H,N,hd)
    kb = np.ascontiguousarray(np.transpose(kp, (1, 2, 3, 0)))  # (K,H,hd,M)
    vb = np.ascontiguousarray(np.transpose(vp, (1, 2, 0, 3)))  # (K,H,M,hd)
    scores = (qb @ kb) / np.float32(np.sqrt(hd))  # (K,H,N,M)
    scores = scores - scores.max(-1, keepdims=True)
    e = np.exp(scores)
    attn = e / e.sum(-1, keepdims=True)
    ao = np.ascontiguousarray(
        np.transpose(attn @ vb, (2, 0, 1, 3))  # (N,K,H,hd)
    ).reshape(N, K, E)
    ao = ao @ ow.T + ob
    return (ao + init_fv).astype(np.float32)


# revision 10
# speedup vs baseline: 2.1564x; 2.1564x over previous
"""AttentionGuidedInterpolation kernel for 8 Trainium2 NeuronCores.

Device (Bass/Tile, SPMD x8): the compute-heavy similarity search —
64 gram matrices (128-dim features, 1024x1024 each, 17.2 GFLOP) on the
TensorEngine, PSUM->SBUF bf16 downcast on the Activation engine, and
top-8 row search via the DVE Max8/MaxIndex instructions. Each core
handles 8 of the 64 independent (slice, batch) units.

The per-core schedule is DVE-roofline-bound: CoreSim shows the DVE at
94.6% occupancy with zero gaps (Max + MaxIndex are mandatory full-row
scans at 1 elem/lane/cycle; neither supports the 2x packed perf modes,
there is no fused max-with-index instruction, and no other engine can
pre-reduce a row for it). PE sits at 18%, Act at 52% — the algorithmic
floor for exact per-row top-k on this hardware.

Dispatch: run_bass_kernel_spmd under axon re-builds and re-jits its
shard_map wrapper on every call (new closure -> jit cache miss -> full
XLA retrace + compile + neuronx cache lookup, several hundred ms of
pure host/RPC overhead per call). We inline the same bass2jax lowering
it uses (_bass_exec_p custom call on the 8-device mesh) but cache the
jitted executable, pre-stage inputs asynchronously, and create the
donated output buffers on-device, so the per-call cost is one blocking
execute round-trip. Falls back to run_bass_kernel_spmd, then to a
numpy top-k, if the internal API is unavailable.

Host (numpy): index-weighted neighbor combine, grid samples, and the
tiny 4-token attention — cheap glue driven by the device-computed
indices, overlapped with the device call.
"""

import sys
import threading
import time

if "/opt/trn_rl_repo" not in sys.path:
    sys.path.insert(0, "/opt/trn_rl_repo")

import numpy as np

TOP_K = 5
R = 1
NUM_HEADS = 8
N, C, D, H, W, K = 4, 128, 16, 32, 32, 8192
S, L = D, H * W  # 16 slices, 1024 positions per slice
N_CORES = 8
UPC = (S * N) // N_CORES  # units per core = 8

_cache = {}


ENC_M = float(3 * 2**22)  # magic round-to-integer constant (ulp = 1)

# Device algorithm. "scan" (Max + MaxIndex, two full DVE scans) measures
# faster on silicon than every variant of the single-scan encoded top-8
# ("enc"): the real DVE runs ~1.47 GHz (vs ~0.9 GHz in the cost model),
# while the enc path's extra 1-row accumulate matmuls pay a PE
# weight-swap penalty ~2.5x the model (89 vs 109 us/exec, measured
# differentially at matched rep count). "enc" is kept as a validated
# alternative — it passes the decode sanity check on HW.
ALGO = "scan"


def _build_bass(reps=1, algo="enc"):
    """Build the gram + top-k program.

    algo="enc" (default): single-DVE-scan encoded top-8. The Act engine
    rewrites each PSUM gram bank in place as fl(16*sim + M) — the f32
    add against M = 3*2^22 (ulp 1) rounds the similarity to an integer
    grid (quantum 1/16, finer than the bf16 ranking it replaces). Two
    cheap 1-row accumulate matmuls then add -M (exact by Sterbenz) and
    (1023-j)/1024 (exact: |Q| <= 2^13 leaves 10 mantissa bits for the
    fraction; every k/1024 is exact in fp16). One DVE max8 over the
    encoded PSUM returns the top-8 values with their column indices
    embedded in the fraction — no max_index scan. This halves the DVE
    work that bounds the 2-scan variant.

    algo="scan": the classic exact path (bf16 downcast + DVE Max +
    MaxIndex) — the runtime fallback if the encoded path's PSUM
    accumulate semantics differ on silicon (detected by the decode
    sanity check in _run_device_topk).

    reps > 1 unrolls the identical per-core workload `reps` times (only
    rep 0 stores outputs) — used by test.py to measure steady-state
    per-execution HW time differentially, cancelling the axon dispatch
    round-trip that dwarfs a single execution.
    """
    import concourse.mybir as mybir
    from concourse import bacc, tile
    from concourse._compat import get_trn_type

    f32 = mybir.dt.float32
    bf16 = mybir.dt.bfloat16
    f16 = mybir.dt.float16
    u16 = mybir.dt.uint16

    nc = bacc.Bacc(
        get_trn_type(),
        target_bir_lowering=False,
        debug=False,
        num_devices=N_CORES,
    )
    sl_in = nc.dram_tensor("sl", [UPC, 128, L], bf16, kind="ExternalInput")

    if algo == "enc":
        negM_in = nc.dram_tensor("negM", [1, L], bf16, kind="ExternalInput")
        frac_in = nc.dram_tensor("frac", [1, L], f16, kind="ExternalInput")
        enc_out = nc.dram_tensor("enc", [UPC, L, TOP_K], f32, kind="ExternalOutput")
        with tile.TileContext(nc) as tc:
            with (
                tc.tile_pool(name="sb", bufs=3) as pool,
                tc.tile_pool(name="cstp", bufs=6) as cstpool,
                tc.tile_pool(name="mxp", bufs=8) as mxpool,
                tc.tile_pool(name="ps", bufs=4, space="PSUM") as pp,
            ):
                negM = cstpool.tile([1, L], bf16, tag="negM")
                nc.sync.dma_start(out=negM[:], in_=negM_in[:])
                frac = cstpool.tile([1, L], f16, tag="frac")
                nc.sync.dma_start(out=frac[:], in_=frac_in[:])
                ones_bf = cstpool.tile([1, 128], bf16, tag="onesb")
                nc.vector.memset(ones_bf[:], 1.0)
                ones_f16 = cstpool.tile([1, 128], f16, tag="onesh")
                nc.vector.memset(ones_f16[:], 1.0)
                biasM = cstpool.tile([128, 1], f32, tag="biasM")
                nc.vector.memset(biasM[:], ENC_M)
                for rep in range(reps):
                    for u in range(UPC):
                        sl_t = pool.tile([128, L], bf16, tag="sl")
                        nc.sync.dma_start(out=sl_t[:], in_=sl_in[u])
                        for lt in range(L // 128):
                            ps = pp.tile([128, L], f32, tag="ps")
                            lhsT = sl_t[:, lt * 128 : (lt + 1) * 128]
                            for h in (0, 1):
                                sl_h = sl_t[:, h * 512 : (h + 1) * 512]
                                p = ps[:, h * 512 : (h + 1) * 512]
                                nc.tensor.matmul(p, lhsT, sl_h)
                                nc.scalar.activation(
                                    p,
                                    p,
                                    mybir.ActivationFunctionType.Identity,
                                    bias=biasM[:],
                                    scale=16.0,
                                )
                                nc.tensor.matmul(
                                    p,
                                    ones_bf[:],
                                    negM[:, h * 512 : (h + 1) * 512],
                                    start=False,
                                    stop=False,
                                    skip_group_check=True,
                                )
                                nc.tensor.matmul(
                                    p,
                                    ones_f16[:],
                                    frac[:, h * 512 : (h + 1) * 512],
                                    start=False,
                                    stop=True,
                                    skip_group_check=True,
                                )
                            mx = mxpool.tile([128, 8], f32, tag="mx")
                            nc.vector.max(mx[:], ps[:])
                            if rep == 0:
                                nc.sync.dma_start(
                                    out=enc_out[u, lt * 128 : (lt + 1) * 128, :],
                                    in_=mx[:, 0:TOP_K],
                                )
        nc.compile()
        return nc

    idxs_out = nc.dram_tensor("idxs", [UPC, L, TOP_K], u16, kind="ExternalOutput")
    with tile.TileContext(nc) as tc:
        with (
            tc.tile_pool(name="sb", bufs=3) as pool,
            tc.tile_pool(name="mxp", bufs=8) as mxpool,
            tc.tile_pool(name="simp", bufs=4) as simpool,
            tc.tile_pool(name="ps", bufs=3, space="PSUM") as pp,
        ):
            for rep in range(reps):
                for u in range(UPC):
                    sl_t = pool.tile([128, L], bf16, tag="sl")
                    nc.sync.dma_start(out=sl_t[:], in_=sl_in[u])
                    for lt in range(L // 128):
                        ps = pp.tile([128, L], f32, tag="ps")
                        lhsT = sl_t[:, lt * 128 : (lt + 1) * 128]
                        # two matmuls: a PSUM bank holds 512 fp32 per partition
                        nc.tensor.matmul(ps[:, 0:512], lhsT, sl_t[:, 0:512])
                        nc.tensor.matmul(ps[:, 512:1024], lhsT, sl_t[:, 512:1024])
                        # bf16 ranking copy: sim values only rank neighbors;
                        # the result is insensitive to rank jitter beyond the
                        # (always exact) self-match because its 1/1e-5 weight
                        # dominates the index-distance softmax.
                        sim_bf = simpool.tile([128, L], bf16, tag="sim")
                        nc.scalar.copy(sim_bf[:, 0:512], ps[:, 0:512])
                        nc.scalar.copy(sim_bf[:, 512:1024], ps[:, 512:1024])
                        mx = mxpool.tile([128, 8], bf16, tag="mx")
                        ix = mxpool.tile([128, 8], u16, tag="ix")
                        nc.vector.max(mx[:], sim_bf[:])
                        nc.vector.max_index(ix[:], mx[:], sim_bf[:])
                        if rep == 0:
                            nc.sync.dma_start(
                                out=idxs_out[u, lt * 128 : (lt + 1) * 128, :],
                                in_=ix[:, 0:TOP_K],
                            )
    nc.compile()
    return nc


class _CachedRunner:
    """bass2jax axon dispatch with a cached jitted executable.

    Mirrors what run_bass_kernel_spmd does under axon (the _bass_exec_p
    custom call inside a shard_map over the 8-core mesh) but builds the
    jit exactly once, so repeat calls skip the retrace/compile.
    """

    def __init__(self, nc):
        import jax
        import jax.numpy as jnp
        from jax.sharding import Mesh, NamedSharding, PartitionSpec

        try:
            from jax.experimental.shard_map import shard_map
        except ImportError:  # newer jax
            from jax import shard_map

        import concourse.mybir as mybir
        from concourse.bass2jax import (
            _bass_exec_p,
            install_neuronx_cc_hook,
            partition_id_tensor,
        )

        install_neuronx_cc_hook()
        self.jax = jax
        self.nc = nc

        partition_name = (
            nc.partition_id_tensor.name if nc.partition_id_tensor else None
        )
        in_names, out_names, out_avals, zero_shapes = [], [], [], []
        for alloc in nc.m.functions[0].allocations:
            if not isinstance(alloc, mybir.MemoryLocationSet):
                continue
            name = alloc.memorylocations[0].name
            if alloc.kind == "ExternalInput":
                if name != partition_name:
                    in_names.append(name)
            elif alloc.kind == "ExternalOutput":
                shape = tuple(alloc.tensor_shape)
                dtype = mybir.dt.np(alloc.dtype)
                out_names.append(name)
                out_avals.append(jax.core.ShapedArray(shape, dtype))
                zero_shapes.append((shape, dtype))
        n_params, n_outs = len(in_names), len(out_avals)
        self.in_names = in_names
        self.out_names = out_names
        in_names_all = in_names + out_names
        if partition_name is not None:
            in_names_all.append(partition_name)

        def _body(*args):
            ops = list(args)
            if partition_name is not None:
                ops.append(partition_id_tensor())
            return tuple(
                _bass_exec_p.bind(
                    *ops,
                    out_avals=tuple(out_avals),
                    in_names=tuple(in_names_all),
                    out_names=tuple(out_names),
                    lowering_input_output_aliases=(),
                    sim_require_finite=True,
                    sim_require_nnan=True,
                    nc=nc,
                )
            )

        devices = jax.devices()[:N_CORES]
        assert len(devices) == N_CORES
        mesh = Mesh(np.asarray(devices), ("core",))
        self.spec = NamedSharding(mesh, PartitionSpec("core"))
        self.sharded = jax.jit(
            shard_map(
                _body,
                mesh=mesh,
                in_specs=(PartitionSpec("core"),) * (n_params + n_outs),
                out_specs=(PartitionSpec("core"),) * n_outs,
                check_rep=False,
            ),
            donate_argnums=tuple(range(n_params, n_params + n_outs)),
            keep_unused=True,
        )
        # Donated output buffers, created on-device (no H2D bytes).
        self.mkzeros = jax.jit(
            lambda: tuple(
                jnp.zeros((N_CORES * s[0], *s[1:]), d) for s, d in zero_shapes
            ),
            out_shardings=(self.spec,) * n_outs,
        )

    def stage(self, in_map):
        """Async H2D of global (cores-concatenated) inputs + donated outputs.

        in_map: {name: global array with axis 0 = n_cores * per_core_dim0}.
        """
        dins = [self.jax.device_put(in_map[n], self.spec) for n in self.in_names]
        zs = self.mkzeros()
        return dins, zs

    def execute(self, dins, zs, block=True):
        """Dispatch the NEFF; returns (device outputs, blocking span ns)."""
        t0 = time.perf_counter()
        out = self.sharded(*dins, *zs)
        if block:
            self.jax.block_until_ready(out)
        span = (time.perf_counter() - t0) * 1e9
        return out, span

    def fetch(self, out):
        """D2H with all shard transfers in flight before any blocks."""
        arrs = []
        for o in out:
            shards = o.addressable_shards
            for s_ in shards:
                s_.data.copy_to_host_async()
            arrs.append(
                np.concatenate([np.asarray(s_.data) for s_ in shards], axis=0)
            )
        return dict(zip(self.out_names, arrs))


def _get_runner(reps=1, algo="enc"):
    key = ("runner", reps, algo)
    if key not in _cache:
        _cache[key] = _CachedRunner(_build_bass(reps=reps, algo=algo))
    return _cache[key]


def _enc_consts():
    """Per-core const rows for the encoded path, replicated across cores."""
    import ml_dtypes

    negM = np.full((N_CORES, L), -ENC_M, np.float32).astype(ml_dtypes.bfloat16)
    frac = np.tile(
        ((1023.0 - np.arange(L)) / 1024.0).astype(np.float16)[None, :], (N_CORES, 1)
    )
    return negM, frac


def _decode_enc(enc):
    """enc (64, L, 5) f32 -> idx int64 + sanity flag.

    enc = Q + (1023-j)/1024 with Q = round(16*sim) an integer: exact in
    f32 for |Q| < 2^13, so the decode recovers j exactly. Sanity: the
    top-1 of a gram row is its own diagonal (self-similarity dominates
    by ~8 quantization sigmas); if the PSUM round-trip behaved
    differently on silicon the fractions collapse and this check fails.
    """
    Q = np.floor(enc)
    j = 1023 - np.rint((enc - Q) * 1024.0).astype(np.int64)
    ok = bool((j >= 0).all() and (j <= 1023).all())
    if ok:
        diag = np.arange(L)[None, :]
        ok = float(np.mean(j[:, :, 0] == diag)) > 0.999
    return j, ok


def _host_topk(sl_full):
    """Numpy fallback: exact gram + top-5 (jax tie-break: value desc, index asc)."""
    slb = sl_full.reshape(S * N, C, L)
    sim = np.matmul(np.transpose(slb, (0, 2, 1)), slb).reshape(S, N, L, L)
    part = np.argpartition(-sim, TOP_K, axis=-1)[..., :TOP_K]
    pvals = np.take_along_axis(sim, part, axis=-1)
    order = np.lexsort((part, -pvals), axis=-1)
    idx = np.take_along_axis(part, order, axis=-1)
    return idx.astype(np.int64)  # (S,N,L,5)


def _to_bf16_units(sl_full):
    import ml_dtypes

    return np.ascontiguousarray(sl_full.reshape(S * N, C, L)).astype(
        ml_dtypes.bfloat16
    )


def _run_device_topk(sl_full):
    """sl_full: (S, N, C, L) f32. Returns idx (S,N,L,5) int64 via 8 cores."""
    sl_units = _to_bf16_units(sl_full)
    t0 = time.perf_counter()
    idx = None
    if ALGO == "enc" and not _cache.get("enc_bad"):
        try:  # encoded single-scan path
            runner = _get_runner(reps=1, algo="enc")
            negM, frac = _enc_consts()
            dins, zs = runner.stage({"sl": sl_units, "negM": negM, "frac": frac})
            out, span = runner.execute(dins, zs)
            res = runner.fetch(out)
            _cache["exec_span_ns"] = span
            j, ok = _decode_enc(res["enc"].reshape(S * N, L, TOP_K))
            if ok:
                idx = j
            else:  # silicon disagreed with the PSUM encode round-trip
                _cache["enc_bad"] = True
        except Exception:  # pragma: no cover - harness-proofing
            _cache["enc_bad"] = True
    if idx is None:
        try:  # exact 2-scan DVE path
            runner = _get_runner(reps=1, algo="scan")
            dins, zs = runner.stage({"sl": sl_units})
            out, span = runner.execute(dins, zs)
            res = runner.fetch(out)
            _cache["exec_span_ns"] = span
            idx = res["idxs"]
        except Exception:  # pragma: no cover
            from concourse.bass_utils import run_bass_kernel_spmd

            if "nc" not in _cache:
                _cache["nc"] = _build_bass(reps=1, algo="scan")
            in_maps = [
                {"sl": np.ascontiguousarray(sl_units[c * UPC : (c + 1) * UPC])}
                for c in range(N_CORES)
            ]
            out = run_bass_kernel_spmd(_cache["nc"], in_maps, list(range(N_CORES)))
            idx = np.concatenate(
                [np.asarray(out.results[c]["idxs"]) for c in range(N_CORES)], 0
            )
    _cache["last_device_ns"] = (time.perf_counter() - t0) * 1e9
    idx = np.clip(idx.reshape(S, N, L, TOP_K).astype(np.int64), 0, L - 1)
    return idx


# ---------------- numpy ports of the reference glue ----------------


def _unnorm(g, size):
    return ((g + 1.0) * size - 1.0) / 2.0


def _grid_sample_3d(fm, grid, mode, fmt=None):
    # fm: (N,C,Dd,Hh,Ww); grid: (N,P,3) last dim (x->W, y->H, z->D)
    # fmt: optional precomputed voxel-major view (N, D*H*W, C)
    n_, c_, d_, h_, w_ = fm.shape
    if fmt is None:
        fmt = np.ascontiguousarray(
            np.transpose(fm, (0, 2, 3, 4, 1)).reshape(n_, d_ * h_ * w_, c_)
        )
    ix = _unnorm(grid[..., 0], w_)
    iy = _unnorm(grid[..., 1], h_)
    iz = _unnorm(grid[..., 2], d_)

    def fetch(z, y, x):
        valid = (z >= 0) & (z < d_) & (y >= 0) & (y < h_) & (x >= 0) & (x < w_)
        lin = (
            np.clip(z, 0, d_ - 1) * (h_ * w_)
            + np.clip(y, 0, h_ - 1) * w_
            + np.clip(x, 0, w_ - 1)
        )
        v = np.take_along_axis(fmt, lin[..., None], axis=1)
        v[~valid] = 0.0
        return v

    if mode == "nearest":
        return fetch(
            np.round(iz).astype(np.int64),
            np.round(iy).astype(np.int64),
            np.round(ix).astype(np.int64),
        )
    x0 = np.floor(ix)
    y0 = np.floor(iy)
    z0 = np.floor(iz)
    tx, ty, tz = ix - x0, iy - y0, iz - z0
    x0i, y0i, z0i = x0.astype(np.int64), y0.astype(np.int64), z0.astype(np.int64)
    out = np.zeros(grid.shape[:-1] + (c_,), fm.dtype)
    for dz in (0, 1):
        for dy in (0, 1):
            for dx in (0, 1):
                wgt = (
                    (tz if dz else 1.0 - tz)
                    * (ty if dy else 1.0 - ty)
                    * (tx if dx else 1.0 - tx)
                ).astype(np.float32)
                out += fetch(z0i + dz, y0i + dy, x0i + dx) * wgt[..., None]
    return out  # (N,P,C)


def _nearest_lin(grid, d_, h_, w_):
    """Shared nearest-voxel linear indices + validity for a (N,P,3) grid."""
    ix = _unnorm(grid[..., 0], w_)
    iy = _unnorm(grid[..., 1], h_)
    iz = _unnorm(grid[..., 2], d_)
    z = np.round(iz).astype(np.int64)
    y = np.round(iy).astype(np.int64)
    x = np.round(ix).astype(np.int64)
    valid = (z >= 0) & (z < d_) & (y >= 0) & (y < h_) & (x >= 0) & (x < w_)
    lin = (
        np.clip(z, 0, d_ - 1) * (h_ * w_)
        + np.clip(y, 0, h_ - 1) * w_
        + np.clip(x, 0, w_ - 1)
    )
    return lin, valid


def _fetch_lin(fmt, lin, valid):
    v = np.take_along_axis(fmt, lin[..., None], axis=1)
    v[~valid] = 0.0
    return v


def _find_neighbor_coords(xyz_hr, fm_shape, r=R):
    d_, h_, w_ = fm_shape[-3:]
    scale = np.array([d_ - 1, h_ - 1, w_ - 1], np.float32)
    g = np.floor((xyz_hr + 1.0) / 2.0 * scale).astype(np.float32)
    steps = np.linspace(-float(r), float(r), 2 * r + 1).astype(np.float32)
    dh, dv = steps * np.float32(2.0 / h_), steps * np.float32(2.0 / w_)
    # mdi == 0 for these shapes (D=16 smallest)
    d2 = np.stack(np.meshgrid(dh, dv, indexing="ij"), -1).reshape(1, 1, -1, 2)
    nc2 = g[..., 1:][:, :, None, :] + d2
    fixed = np.broadcast_to(g[..., 0:1][:, :, None, :], nc2.shape[:3] + (1,))
    ncrd = np.concatenate([fixed, nc2], -1).astype(np.float32)
    return ncrd / scale * 2.0 - 1.0  # (N,K,A,3)


def kernel(**inputs):
    fm = np.asarray(inputs["feature_map"], np.float32)
    xyz = np.asarray(inputs["xyz_hr"], np.float32)
    Wq = np.asarray(inputs["Wq"], np.float32)
    bq = np.asarray(inputs["bq"], np.float32)
    Wk = np.asarray(inputs["Wk"], np.float32)
    bk = np.asarray(inputs["bk"], np.float32)
    Wv = np.asarray(inputs["Wv"], np.float32)
    bv = np.asarray(inputs["bv"], np.float32)
    ipw = np.asarray(inputs["in_proj_w"], np.float32)
    ipb = np.asarray(inputs["in_proj_b"], np.float32)
    ow = np.asarray(inputs["out_w"], np.float32)
    ob = np.asarray(inputs["out_b"], np.float32)

    # ---- similarity search: gram + top-8 on the 8 NeuronCores ----
    sl_full = np.ascontiguousarray(
        np.transpose(fm, (2, 0, 1, 3, 4)).reshape(S, N, C, L)
    )

    # Run the device top-k concurrently with the host-side sampling work
    # that does not depend on it (bilinear init_fv, neighbor coords, nf).
    dev = {}

    def _dev_worker():
        try:
            dev["idx"] = _run_device_topk(sl_full)  # (S,N,L,5)
        except Exception as e:  # device path unavailable -> host fallback
            dev["err"] = e

    th = threading.Thread(target=_dev_worker)
    th.start()

    # ---- device-independent sampling work (overlapped with the device call) ----
    fmt_fm = np.ascontiguousarray(
        np.transpose(fm, (0, 2, 3, 4, 1)).reshape(N, D * H * W, C)
    )
    init_fv = _grid_sample_3d(fm, xyz[..., ::-1], "bilinear", fmt=fmt_fm)  # (N,K,C)
    ncrd = _find_neighbor_coords(xyz, fm.shape)  # (N,K,A,3)
    A = ncrd.shape[2]
    grid_n = ncrd.reshape(N, K * A, 3)[..., ::-1]
    lin_n, valid_n = _nearest_lin(grid_n, D, H, W)  # shared by nf and sf
    nf = _fetch_lin(fmt_fm, lin_n, valid_n)
    rd = np.linalg.norm(
        xyz[:, :, None, None, :] - ncrd[:, :, None, :, :], axis=-1
    ).astype(np.float32)
    rw = 1.0 / (rd + np.float32(1e-6))
    rw = (rw / rw.sum(-1, keepdims=True)).reshape(N, K, 1, A)  # (N,K,1,A)

    th.join()
    idx = dev.get("idx")
    if idx is None:
        idx = _host_topk(sl_full)

    # ---- index-weighted neighbor combine (host) ----
    featsT = np.ascontiguousarray(np.transpose(sl_full, (0, 1, 3, 2))).reshape(
        S * N, L, C
    )
    dist = np.abs(idx - np.arange(L)[None, None, :, None]).astype(
        np.float32
    ) + np.float32(1e-5)
    w = 1.0 / dist
    w = (w / w.sum(-1, keepdims=True)).astype(np.float32).reshape(S * N, L, TOP_K)
    idx_f = idx.reshape(S * N, L, TOP_K)
    g5 = np.take_along_axis(
        featsT, idx_f.reshape(S * N, L * TOP_K, 1), axis=1
    ).reshape(S * N, L, TOP_K, C)
    wa_lc = (w[:, :, None, :] @ g5).reshape(S * N, L, C)
    # Direct permutation of wa_lc (S,N,L,C) to the voxel-major layout the
    # nearest-sample needs — equivalent to building sim_feats=(N,C,D,H,W) and
    # re-transposing, but with one copy instead of two. Index algebra:
    # sim_feats[n,c,d,h,w] = wa[4n + c//32, (c%32)//8, (c%8)*16 + d, h*32+w].
    sim_fmt = np.ascontiguousarray(
        wa_lc.reshape(4, 4, 4, L, 8, 16).transpose(0, 5, 3, 1, 2, 4)
    ).reshape(N, D * H * W, C)

    sf = _fetch_lin(sim_fmt, lin_n, valid_n)
    # comb = ((nf_v*rw).sum(2)+(sf_v*rw).sum(2))/2 == ((nf_v+sf_v)*rw).sum(2)/2,
    # so add before the raw (N,C,P)->(N,K,A,C) view and weight once. The
    # torch view maps (k,a,c) -> s[n, (k%64)*A*C + a*C + c, k//64] (since
    # P = K*A = 64*A*C here), so contract from that strided view directly
    # instead of materialising the 300MB (N,C,P) transpose copy.
    sr = (nf + sf).reshape(N, 64, A, C, 128)  # [n, k%64, a, c, k//64]
    rwr = rw.reshape(N, 128, 64, A)  # [n, k//64, k%64, a]
    comb = np.einsum("nqma,nmafq->nqmf", rwr, sr, optimize=True).reshape(
        N, K, C
    ) / np.float32(2.0)

    # ---- projections + 4-token attention (seq axis = N, batch = K) ----
    q = init_fv @ Wq.T + bq
    k = comb @ Wk.T + bk
    v = comb @ Wv.T + bv
    E = C
    hd = E // NUM_HEADS
    qp = (q @ ipw[:E].T + ipb[:E]).reshape(N, K, NUM_HEADS, hd)
    kp = (k @ ipw[E : 2 * E].T + ipb[E : 2 * E]).reshape(N, K, NUM_HEADS, hd)
    vp = (v @ ipw[2 * E :].T + ipb[2 * E :]).reshape(N, K, NUM_HEADS, hd)
    qb = np.ascontiguousarray(np.transpose(qp, (1, 2, 0, 3)))  # (K,H,N,hd)
    kb = np.ascontiguousarray(np.transpose(kp, (1, 2, 3, 0)))  # (K,H,hd,M)
    vb = np.ascontiguousarray(np.transpose(vp, (1, 2, 0, 3)))  # (K,H,M,hd)
    scores = (qb @ kb) / np.float32(np.sqrt(hd))  # (K,H,N,M)
    scores = scores - scores.max(-1, keepdims=True)
    e = np.exp(scores)
    attn = e / e.sum(-1, keepdims=True)
    ao = np.ascontiguousarray(
        np.transpose(attn @ vb, (2, 0, 1, 3))  # (N,K,H,hd)
    ).reshape(N, K, E)
    ao = ao @ ow.T + ob
    return (ao + init_fv).astype(np.float32)


# revision 13
# speedup vs baseline: 2.4629x; 1.1421x over previous
"""AttentionGuidedInterpolation kernel for 8 Trainium2 NeuronCores.

Device (Bass/Tile, SPMD x8): the compute-heavy similarity search —
64 gram matrices (128-dim features, 1024x1024 each, 17.2 GFLOP) on the
TensorEngine, PSUM->SBUF bf16 downcast on the Activation engine, and
top-8 row search via the DVE Max8/MaxIndex instructions. Each core
handles 8 of the 64 independent (slice, batch) units.

The per-core schedule is DVE-roofline-bound: CoreSim shows the DVE at
94.6% occupancy with zero gaps (Max + MaxIndex are mandatory full-row
scans at 1 elem/lane/cycle; neither supports the 2x packed perf modes,
there is no fused max-with-index instruction, and no other engine can
pre-reduce a row for it). PE sits at 18%, Act at 52% — the algorithmic
floor for exact per-row top-k on this hardware.

Dispatch: run_bass_kernel_spmd under axon re-builds and re-jits its
shard_map wrapper on every call (new closure -> jit cache miss -> full
XLA retrace + compile + neuronx cache lookup, several hundred ms of
pure host/RPC overhead per call). We inline the same bass2jax lowering
it uses (_bass_exec_p custom call on the 8-device mesh) but cache the
jitted executable, pre-stage inputs asynchronously, and create the
donated output buffers on-device, so the per-call cost is one blocking
execute round-trip. Falls back to run_bass_kernel_spmd, then to a
numpy top-k, if the internal API is unavailable.

Host (numpy): index-weighted neighbor combine, grid samples, and the
tiny 4-token attention — cheap glue driven by the device-computed
indices, overlapped with the device call.
"""

import sys
import threading
import time

if "/opt/trn_rl_repo" not in sys.path:
    sys.path.insert(0, "/opt/trn_rl_repo")

import numpy as np

TOP_K = 5
R = 1
NUM_HEADS = 8
N, C, D, H, W, K = 4, 128, 16, 32, 32, 8192
S, L = D, H * W  # 16 slices, 1024 positions per slice
N_CORES = 8
UPC = (S * N) // N_CORES  # units per core = 8

_cache = {}


ENC_M = float(3 * 2**22)  # magic round-to-integer constant (ulp = 1)

# Device algorithm. "hybrid" runs 6 of each unit's 8 row-tiles on the
# exact 2-scan path (Max + MaxIndex) and 2 (ENC_LTS) on the single-scan
# encoded path: the scan path is DVE-bound with PE/Act mostly idle, the
# enc path is PE/Act-bound with DVE half-idle, so interleaving them
# both trims total DVE work and fills the DVE dependency bubbles
# between each scan tile's max -> max_index pair. Measured head-to-head
# in shared tunnel-drift windows: hybrid ~69 us vs scan ~118 us vs
# all-enc ~109+ us per execution. "scan" remains the fallback if the
# enc tiles' PSUM encode round-trip misbehaves (decode sanity check).
ALGO = "hybrid"
ENC_LTS = (2, 5)


def _build_bass(reps=1, algo="enc"):
    """Build the gram + top-k program.

    algo="enc" (default): single-DVE-scan encoded top-8. The Act engine
    rewrites each PSUM gram bank in place as fl(16*sim + M) — the f32
    add against M = 3*2^22 (ulp 1) rounds the similarity to an integer
    grid (quantum 1/16, finer than the bf16 ranking it replaces). Two
    cheap 1-row accumulate matmuls then add -M (exact by Sterbenz) and
    (1023-j)/1024 (exact: |Q| <= 2^13 leaves 10 mantissa bits for the
    fraction; every k/1024 is exact in fp16). One DVE max8 over the
    encoded PSUM returns the top-8 values with their column indices
    embedded in the fraction — no max_index scan. This halves the DVE
    work that bounds the 2-scan variant.

    algo="scan": the classic exact path (bf16 downcast + DVE Max +
    MaxIndex) — the runtime fallback if the encoded path's PSUM
    accumulate semantics differ on silicon (detected by the decode
    sanity check in _run_device_topk).

    reps > 1 unrolls the identical per-core workload `reps` times (only
    rep 0 stores outputs) — used by test.py to measure steady-state
    per-execution HW time differentially, cancelling the axon dispatch
    round-trip that dwarfs a single execution.
    """
    import concourse.mybir as mybir
    from concourse import bacc, tile
    from concourse._compat import get_trn_type

    f32 = mybir.dt.float32
    bf16 = mybir.dt.bfloat16
    f16 = mybir.dt.float16
    u16 = mybir.dt.uint16

    nc = bacc.Bacc(
        get_trn_type(),
        target_bir_lowering=False,
        debug=False,
        num_devices=N_CORES,
    )
    sl_in = nc.dram_tensor("sl", [UPC, 128, L], bf16, kind="ExternalInput")

    if algo == "hybrid":
        negM_in = nc.dram_tensor("negM", [1, L], bf16, kind="ExternalInput")
        frac_in = nc.dram_tensor("frac", [1, L], f16, kind="ExternalInput")
        idxs_out = nc.dram_tensor("idxs", [UPC, L, TOP_K], u16, kind="ExternalOutput")
        enc_out = nc.dram_tensor("enc", [UPC, L, TOP_K], f32, kind="ExternalOutput")
        with tile.TileContext(nc) as tc:
            with (
                tc.tile_pool(name="sb", bufs=3) as pool,
                tc.tile_pool(name="cstp", bufs=8) as cstpool,
                tc.tile_pool(name="simp", bufs=4) as simpool,
                tc.tile_pool(name="mxp", bufs=8) as mxpool,
                tc.tile_pool(name="ps", bufs=4, space="PSUM") as pp,
            ):
                negM = cstpool.tile([1, L], bf16, tag="negM")
                nc.sync.dma_start(out=negM[:], in_=negM_in[:])
                frac = cstpool.tile([1, L], f16, tag="frac")
                nc.sync.dma_start(out=frac[:], in_=frac_in[:])
                ones_bf = cstpool.tile([1, 128], bf16, tag="onesb")
                nc.vector.memset(ones_bf[:], 1.0)
                ones_f16 = cstpool.tile([1, 128], f16, tag="onesh")
                nc.vector.memset(ones_f16[:], 1.0)
                biasM = cstpool.tile([128, 1], f32, tag="biasM")
                nc.vector.memset(biasM[:], ENC_M)
                for rep in range(reps):
                    for u in range(UPC):
                        sl_t = pool.tile([128, L], bf16, tag="sl")
                        nc.sync.dma_start(out=sl_t[:], in_=sl_in[u])
                        for lt in range(L // 128):
                            ps = pp.tile([128, L], f32, tag="ps")
                            lhsT = sl_t[:, lt * 128 : (lt + 1) * 128]
                            if lt in ENC_LTS:
                                for h in (0, 1):
                                    nc.tensor.matmul(
                                        ps[:, h * 512 : (h + 1) * 512],
                                        lhsT,
                                        sl_t[:, h * 512 : (h + 1) * 512],
                                    )
                                for h in (0, 1):
                                    nc.scalar.activation(
                                        ps[:, h * 512 : (h + 1) * 512],
                                        ps[:, h * 512 : (h + 1) * 512],
                                        mybir.ActivationFunctionType.Identity,
                                        bias=biasM[:],
                                        scale=16.0,
                                    )
                                for h in (0, 1):
                                    nc.tensor.matmul(
                                        ps[:, h * 512 : (h + 1) * 512],
                                        ones_bf[:],
                                        negM[:, h * 512 : (h + 1) * 512],
                                        start=False,
                                        stop=False,
                                        skip_group_check=True,
                                    )
                                for h in (0, 1):
                                    nc.tensor.matmul(
                                        ps[:, h * 512 : (h + 1) * 512],
                                        ones_f16[:],
                                        frac[:, h * 512 : (h + 1) * 512],
                                        start=False,
                                        stop=True,
                                        skip_group_check=True,
                                    )
                                mx = mxpool.tile([128, 8], f32, tag="mxf")
                                nc.vector.max(mx[:], ps[:])
                                if rep == 0:
                                    nc.sync.dma_start(
                                        out=enc_out[u, lt * 128 : (lt + 1) * 128, :],
                                        in_=mx[:, 0:TOP_K],
                                    )
                            else:
                                nc.tensor.matmul(ps[:, 0:512], lhsT, sl_t[:, 0:512])
                                nc.tensor.matmul(
                                    ps[:, 512:1024], lhsT, sl_t[:, 512:1024]
                                )
                                sim_bf = simpool.tile([128, L], bf16, tag="sim")
                                nc.scalar.copy(sim_bf[:, 0:512], ps[:, 0:512])
                                nc.scalar.copy(sim_bf[:, 512:1024], ps[:, 512:1024])
                                mxb = mxpool.tile([128, 8], bf16, tag="mxb")
                                ix = mxpool.tile([128, 8], u16, tag="ix")
                                nc.vector.max(mxb[:], sim_bf[:])
                                nc.vector.max_index(ix[:], mxb[:], sim_bf[:])
                                if rep == 0:
                                    nc.sync.dma_start(
                                        out=idxs_out[u, lt * 128 : (lt + 1) * 128, :],
                                        in_=ix[:, 0:TOP_K],
                                    )
        nc.compile()
        return nc

    if algo == "enc":
        negM_in = nc.dram_tensor("negM", [1, L], bf16, kind="ExternalInput")
        frac_in = nc.dram_tensor("frac", [1, L], f16, kind="ExternalInput")
        enc_out = nc.dram_tensor("enc", [UPC, L, TOP_K], f32, kind="ExternalOutput")
        with tile.TileContext(nc) as tc:
            with (
                tc.tile_pool(name="sb", bufs=3) as pool,
                tc.tile_pool(name="cstp", bufs=6) as cstpool,
                tc.tile_pool(name="mxp", bufs=8) as mxpool,
                tc.tile_pool(name="ps", bufs=4, space="PSUM") as pp,
            ):
                negM = cstpool.tile([1, L], bf16, tag="negM")
                nc.sync.dma_start(out=negM[:], in_=negM_in[:])
                frac = cstpool.tile([1, L], f16, tag="frac")
                nc.sync.dma_start(out=frac[:], in_=frac_in[:])
                ones_bf = cstpool.tile([1, 128], bf16, tag="onesb")
                nc.vector.memset(ones_bf[:], 1.0)
                ones_f16 = cstpool.tile([1, 128], f16, tag="onesh")
                nc.vector.memset(ones_f16[:], 1.0)
                biasM = cstpool.tile([128, 1], f32, tag="biasM")
                nc.vector.memset(biasM[:], ENC_M)
                for rep in range(reps):
                    for u in range(UPC):
                        sl_t = pool.tile([128, L], bf16, tag="sl")
                        nc.sync.dma_start(out=sl_t[:], in_=sl_in[u])
                        for lt in range(L // 128):
                            ps = pp.tile([128, L], f32, tag="ps")
                            lhsT = sl_t[:, lt * 128 : (lt + 1) * 128]
                            for h in (0, 1):
                                sl_h = sl_t[:, h * 512 : (h + 1) * 512]
                                p = ps[:, h * 512 : (h + 1) * 512]
                                nc.tensor.matmul(p, lhsT, sl_h)
                                nc.scalar.activation(
                                    p,
                                    p,
                                    mybir.ActivationFunctionType.Identity,
                                    bias=biasM[:],
                                    scale=16.0,
                                )
                                nc.tensor.matmul(
                                    p,
                                    ones_bf[:],
                                    negM[:, h * 512 : (h + 1) * 512],
                                    start=False,
                                    stop=False,
                                    skip_group_check=True,
                                )
                                nc.tensor.matmul(
                                    p,
                                    ones_f16[:],
                                    frac[:, h * 512 : (h + 1) * 512],
                                    start=False,
                                    stop=True,
                                    skip_group_check=True,
                                )
                            mx = mxpool.tile([128, 8], f32, tag="mx")
                            nc.vector.max(mx[:], ps[:])
                            if rep == 0:
                                nc.sync.dma_start(
                                    out=enc_out[u, lt * 128 : (lt + 1) * 128, :],
                                    in_=mx[:, 0:TOP_K],
                                )
        nc.compile()
        return nc

    idxs_out = nc.dram_tensor("idxs", [UPC, L, TOP_K], u16, kind="ExternalOutput")
    with tile.TileContext(nc) as tc:
        with (
            tc.tile_pool(name="sb", bufs=3) as pool,
            tc.tile_pool(name="mxp", bufs=8) as mxpool,
            tc.tile_pool(name="simp", bufs=4) as simpool,
            tc.tile_pool(name="ps", bufs=3, space="PSUM") as pp,
        ):
            for rep in range(reps):
                for u in range(UPC):
                    sl_t = pool.tile([128, L], bf16, tag="sl")
                    nc.sync.dma_start(out=sl_t[:], in_=sl_in[u])
                    for lt in range(L // 128):
                        ps = pp.tile([128, L], f32, tag="ps")
                        lhsT = sl_t[:, lt * 128 : (lt + 1) * 128]
                        # two matmuls: a PSUM bank holds 512 fp32 per partition
                        nc.tensor.matmul(ps[:, 0:512], lhsT, sl_t[:, 0:512])
                        nc.tensor.matmul(ps[:, 512:1024], lhsT, sl_t[:, 512:1024])
                        # bf16 ranking copy: sim values only rank neighbors;
                        # the result is insensitive to rank jitter beyond the
                        # (always exact) self-match because its 1/1e-5 weight
                        # dominates the index-distance softmax.
                        sim_bf = simpool.tile([128, L], bf16, tag="sim")
                        nc.scalar.copy(sim_bf[:, 0:512], ps[:, 0:512])
                        nc.scalar.copy(sim_bf[:, 512:1024], ps[:, 512:1024])
                        mx = mxpool.tile([128, 8], bf16, tag="mx")
                        ix = mxpool.tile([128, 8], u16, tag="ix")
                        nc.vector.max(mx[:], sim_bf[:])
                        nc.vector.max_index(ix[:], mx[:], sim_bf[:])
                        if rep == 0:
                            nc.sync.dma_start(
                                out=idxs_out[u, lt * 128 : (lt + 1) * 128, :],
                                in_=ix[:, 0:TOP_K],
                            )
    nc.compile()
    return nc


class _CachedRunner:
    """bass2jax axon dispatch with a cached jitted executable.

    Mirrors what run_bass_kernel_spmd does under axon (the _bass_exec_p
    custom call inside a shard_map over the 8-core mesh) but builds the
    jit exactly once, so repeat calls skip the retrace/compile.
    """

    def __init__(self, nc):
        import jax
        import jax.numpy as jnp
        from jax.sharding import Mesh, NamedSharding, PartitionSpec

        try:
            from jax.experimental.shard_map import shard_map
        except ImportError:  # newer jax
            from jax import shard_map

        import concourse.mybir as mybir
        from concourse.bass2jax import (
            _bass_exec_p,
            install_neuronx_cc_hook,
            partition_id_tensor,
        )

        install_neuronx_cc_hook()
        self.jax = jax
        self.nc = nc

        partition_name = (
            nc.partition_id_tensor.name if nc.partition_id_tensor else None
        )
        in_names, out_names, out_avals, zero_shapes = [], [], [], []
        for alloc in nc.m.functions[0].allocations:
            if not isinstance(alloc, mybir.MemoryLocationSet):
                continue
            name = alloc.memorylocations[0].name
            if alloc.kind == "ExternalInput":
                if name != partition_name:
                    in_names.append(name)
            elif alloc.kind == "ExternalOutput":
                shape = tuple(alloc.tensor_shape)
                dtype = mybir.dt.np(alloc.dtype)
                out_names.append(name)
                out_avals.append(jax.core.ShapedArray(shape, dtype))
                zero_shapes.append((shape, dtype))
        n_params, n_outs = len(in_names), len(out_avals)
        self.in_names = in_names
        self.out_names = out_names
        in_names_all = in_names + out_names
        if partition_name is not None:
            in_names_all.append(partition_name)

        def _body(*args):
            ops = list(args)
            if partition_name is not None:
                ops.append(partition_id_tensor())
            return tuple(
                _bass_exec_p.bind(
                    *ops,
                    out_avals=tuple(out_avals),
                    in_names=tuple(in_names_all),
                    out_names=tuple(out_names),
                    lowering_input_output_aliases=(),
                    sim_require_finite=True,
                    sim_require_nnan=True,
                    nc=nc,
                )
            )

        devices = jax.devices()[:N_CORES]
        assert len(devices) == N_CORES
        mesh = Mesh(np.asarray(devices), ("core",))
        self.spec = NamedSharding(mesh, PartitionSpec("core"))
        self.sharded = jax.jit(
            shard_map(
                _body,
                mesh=mesh,
                in_specs=(PartitionSpec("core"),) * (n_params + n_outs),
                out_specs=(PartitionSpec("core"),) * n_outs,
                check_rep=False,
            ),
            donate_argnums=tuple(range(n_params, n_params + n_outs)),
            keep_unused=True,
        )
        # Donated output buffers, created on-device (no H2D bytes).
        self.mkzeros = jax.jit(
            lambda: tuple(
                jnp.zeros((N_CORES * s[0], *s[1:]), d) for s, d in zero_shapes
            ),
            out_shardings=(self.spec,) * n_outs,
        )

    def stage(self, in_map):
        """Async H2D of global (cores-concatenated) inputs + donated outputs.

        in_map: {name: global array with axis 0 = n_cores * per_core_dim0}.
        """
        dins = [self.jax.device_put(in_map[n], self.spec) for n in self.in_names]
        zs = self.mkzeros()
        return dins, zs

    def execute(self, dins, zs, block=True):
        """Dispatch the NEFF; returns (device outputs, blocking span ns)."""
        t0 = time.perf_counter()
        out = self.sharded(*dins, *zs)
        if block:
            self.jax.block_until_ready(out)
        span = (time.perf_counter() - t0) * 1e9
        return out, span

    def fetch(self, out):
        """D2H with all shard transfers in flight before any blocks."""
        arrs = []
        for o in out:
            shards = o.addressable_shards
            for s_ in shards:
                s_.data.copy_to_host_async()
            arrs.append(
                np.concatenate([np.asarray(s_.data) for s_ in shards], axis=0)
            )
        return dict(zip(self.out_names, arrs))


def _get_runner(reps=1, algo="enc"):
    key = ("runner", reps, algo)
    if key not in _cache:
        _cache[key] = _CachedRunner(_build_bass(reps=reps, algo=algo))
    return _cache[key]


def _enc_consts():
    """Per-core const rows for the encoded path, replicated across cores."""
    import ml_dtypes

    negM = np.full((N_CORES, L), -ENC_M, np.float32).astype(ml_dtypes.bfloat16)
    frac = np.tile(
        ((1023.0 - np.arange(L)) / 1024.0).astype(np.float16)[None, :], (N_CORES, 1)
    )
    return negM, frac


def _decode_enc(enc):
    """enc (64, L, 5) f32 -> idx int64 + sanity flag.

    enc = Q + (1023-j)/1024 with Q = round(16*sim) an integer: exact in
    f32 for |Q| < 2^13, so the decode recovers j exactly. Sanity: the
    top-1 of a gram row is its own diagonal (self-similarity dominates
    by ~8 quantization sigmas); if the PSUM round-trip behaved
    differently on silicon the fractions collapse and this check fails.
    """
    Q = np.floor(enc)
    j = 1023 - np.rint((enc - Q) * 1024.0).astype(np.int64)
    ok = bool((j >= 0).all() and (j <= 1023).all())
    if ok:
        diag = np.arange(L)[None, :]
        ok = float(np.mean(j[:, :, 0] == diag)) > 0.999
    return j, ok


def _host_topk(sl_full):
    """Numpy fallback: exact gram + top-5 (jax tie-break: value desc, index asc)."""
    slb = sl_full.reshape(S * N, C, L)
    sim = np.matmul(np.transpose(slb, (0, 2, 1)), slb).reshape(S, N, L, L)
    part = np.argpartition(-sim, TOP_K, axis=-1)[..., :TOP_K]
    pvals = np.take_along_axis(sim, part, axis=-1)
    order = np.lexsort((part, -pvals), axis=-1)
    idx = np.take_along_axis(part, order, axis=-1)
    return idx.astype(np.int64)  # (S,N,L,5)


def _to_bf16_units(sl_full):
    import ml_dtypes

    return np.ascontiguousarray(sl_full.reshape(S * N, C, L)).astype(
        ml_dtypes.bfloat16
    )


def _run_device_topk(sl_full):
    """sl_full: (S, N, C, L) f32. Returns idx (S,N,L,5) int64 via 8 cores."""
    sl_units = _to_bf16_units(sl_full)
    t0 = time.perf_counter()
    idx = None
    if ALGO == "hybrid" and not _cache.get("enc_bad"):
        try:  # mixed scan/enc tiles: merge the two output tensors
            runner = _get_runner(reps=1, algo="hybrid")
            negM, frac = _enc_consts()
            dins, zs = runner.stage({"sl": sl_units, "negM": negM, "frac": frac})
            out, span = runner.execute(dins, zs)
            res = runner.fetch(out)
            _cache["exec_span_ns"] = span
            merged = res["idxs"].reshape(S * N, L, TOP_K).astype(np.int64)
            enc = res["enc"].reshape(S * N, L, TOP_K)
            for lt in ENC_LTS:
                sl_rows = slice(lt * 128, (lt + 1) * 128)
                e = enc[:, sl_rows]
                Q = np.floor(e)
                merged[:, sl_rows] = 1023 - np.rint((e - Q) * 1024.0).astype(
                    np.int64
                )
            ok = bool((merged >= 0).all() and (merged <= 1023).all())
            if ok:
                diag = np.arange(L)[None, :]
                ok = float(np.mean(merged[:, :, 0] == diag)) > 0.999
            if ok:
                idx = merged
            else:
                _cache["enc_bad"] = True
        except Exception:  # pragma: no cover - harness-proofing
            _cache["enc_bad"] = True
    if ALGO == "enc" and not _cache.get("enc_bad"):
        try:  # encoded single-scan path
            runner = _get_runner(reps=1, algo="enc")
            negM, frac = _enc_consts()
            dins, zs = runner.stage({"sl": sl_units, "negM": negM, "frac": frac})
            out, span = runner.execute(dins, zs)
            res = runner.fetch(out)
            _cache["exec_span_ns"] = span
            j, ok = _decode_enc(res["enc"].reshape(S * N, L, TOP_K))
            if ok:
                idx = j
            else:  # silicon disagreed with the PSUM encode round-trip
                _cache["enc_bad"] = True
        except Exception:  # pragma: no cover - harness-proofing
            _cache["enc_bad"] = True
    if idx is None:
        try:  # exact 2-scan DVE path
            runner = _get_runner(reps=1, algo="scan")
            dins, zs = runner.stage({"sl": sl_units})
            out, span = runner.execute(dins, zs)
            res = runner.fetch(out)
            _cache["exec_span_ns"] = span
            idx = res["idxs"]
        except Exception:  # pragma: no cover
            from concourse.bass_utils import run_bass_kernel_spmd

            if "nc" not in _cache:
                _cache["nc"] = _build_bass(reps=1, algo="scan")
            in_maps = [
                {"sl": np.ascontiguousarray(sl_units[c * UPC : (c + 1) * UPC])}
                for c in range(N_CORES)
            ]
            out = run_bass_kernel_spmd(_cache["nc"], in_maps, list(range(N_CORES)))
            idx = np.concatenate(
                [np.asarray(out.results[c]["idxs"]) for c in range(N_CORES)], 0
            )
    _cache["last_device_ns"] = (time.perf_counter() - t0) * 1e9
    idx = np.clip(idx.reshape(S, N, L, TOP_K).astype(np.int64), 0, L - 1)
    return idx


# ---------------- numpy ports of the reference glue ----------------


def _unnorm(g, size):
    return ((g + 1.0) * size - 1.0) / 2.0


def _grid_sample_3d(fm, grid, mode, fmt=None):
    # fm: (N,C,Dd,Hh,Ww); grid: (N,P,3) last dim (x->W, y->H, z->D)
    # fmt: optional precomputed voxel-major view (N, D*H*W, C)
    n_, c_, d_, h_, w_ = fm.shape
    if fmt is None:
        fmt = np.ascontiguousarray(
            np.transpose(fm, (0, 2, 3, 4, 1)).reshape(n_, d_ * h_ * w_, c_)
        )
    ix = _unnorm(grid[..., 0], w_)
    iy = _unnorm(grid[..., 1], h_)
    iz = _unnorm(grid[..., 2], d_)

    def fetch(z, y, x):
        valid = (z >= 0) & (z < d_) & (y >= 0) & (y < h_) & (x >= 0) & (x < w_)
        lin = (
            np.clip(z, 0, d_ - 1) * (h_ * w_)
            + np.clip(y, 0, h_ - 1) * w_
            + np.clip(x, 0, w_ - 1)
        )
        v = np.take_along_axis(fmt, lin[..., None], axis=1)
        v[~valid] = 0.0
        return v

    if mode == "nearest":
        return fetch(
            np.round(iz).astype(np.int64),
            np.round(iy).astype(np.int64),
            np.round(ix).astype(np.int64),
        )
    x0 = np.floor(ix)
    y0 = np.floor(iy)
    z0 = np.floor(iz)
    tx, ty, tz = ix - x0, iy - y0, iz - z0
    x0i, y0i, z0i = x0.astype(np.int64), y0.astype(np.int64), z0.astype(np.int64)
    out = np.zeros(grid.shape[:-1] + (c_,), fm.dtype)
    for dz in (0, 1):
        for dy in (0, 1):
            for dx in (0, 1):
                wgt = (
                    (tz if dz else 1.0 - tz)
                    * (ty if dy else 1.0 - ty)
                    * (tx if dx else 1.0 - tx)
                ).astype(np.float32)
                out += fetch(z0i + dz, y0i + dy, x0i + dx) * wgt[..., None]
    return out  # (N,P,C)


def _nearest_lin(grid, d_, h_, w_):
    """Shared nearest-voxel linear indices + validity for a (N,P,3) grid."""
    ix = _unnorm(grid[..., 0], w_)
    iy = _unnorm(grid[..., 1], h_)
    iz = _unnorm(grid[..., 2], d_)
    z = np.round(iz).astype(np.int64)
    y = np.round(iy).astype(np.int64)
    x = np.round(ix).astype(np.int64)
    valid = (z >= 0) & (z < d_) & (y >= 0) & (y < h_) & (x >= 0) & (x < w_)
    lin = (
        np.clip(z, 0, d_ - 1) * (h_ * w_)
        + np.clip(y, 0, h_ - 1) * w_
        + np.clip(x, 0, w_ - 1)
    )
    return lin, valid


def _fetch_lin(fmt, lin, valid):
    v = np.take_along_axis(fmt, lin[..., None], axis=1)
    v[~valid] = 0.0
    return v


def _find_neighbor_coords(xyz_hr, fm_shape, r=R):
    d_, h_, w_ = fm_shape[-3:]
    scale = np.array([d_ - 1, h_ - 1, w_ - 1], np.float32)
    g = np.floor((xyz_hr + 1.0) / 2.0 * scale).astype(np.float32)
    steps = np.linspace(-float(r), float(r), 2 * r + 1).astype(np.float32)
    dh, dv = steps * np.float32(2.0 / h_), steps * np.float32(2.0 / w_)
    # mdi == 0 for these shapes (D=16 smallest)
    d2 = np.stack(np.meshgrid(dh, dv, indexing="ij"), -1).reshape(1, 1, -1, 2)
    nc2 = g[..., 1:][:, :, None, :] + d2
    fixed = np.broadcast_to(g[..., 0:1][:, :, None, :], nc2.shape[:3] + (1,))
    ncrd = np.concatenate([fixed, nc2], -1).astype(np.float32)
    return ncrd / scale * 2.0 - 1.0  # (N,K,A,3)


def kernel(**inputs):
    fm = np.asarray(inputs["feature_map"], np.float32)
    xyz = np.asarray(inputs["xyz_hr"], np.float32)
    Wq = np.asarray(inputs["Wq"], np.float32)
    bq = np.asarray(inputs["bq"], np.float32)
    Wk = np.asarray(inputs["Wk"], np.float32)
    bk = np.asarray(inputs["bk"], np.float32)
    Wv = np.asarray(inputs["Wv"], np.float32)
    bv = np.asarray(inputs["bv"], np.float32)
    ipw = np.asarray(inputs["in_proj_w"], np.float32)
    ipb = np.asarray(inputs["in_proj_b"], np.float32)
    ow = np.asarray(inputs["out_w"], np.float32)
    ob = np.asarray(inputs["out_b"], np.float32)

    # ---- similarity search: gram + top-8 on the 8 NeuronCores ----
    sl_full = np.ascontiguousarray(
        np.transpose(fm, (2, 0, 1, 3, 4)).reshape(S, N, C, L)
    )

    # Run the device top-k concurrently with the host-side sampling work
    # that does not depend on it (bilinear init_fv, neighbor coords, nf).
    dev = {}

    def _dev_worker():
        try:
            dev["idx"] = _run_device_topk(sl_full)  # (S,N,L,5)
        except Exception as e:  # device path unavailable -> host fallback
            dev["err"] = e

    th = threading.Thread(target=_dev_worker)
    th.start()

    # ---- device-independent sampling work (overlapped with the device call) ----
    fmt_fm = np.ascontiguousarray(
        np.transpose(fm, (0, 2, 3, 4, 1)).reshape(N, D * H * W, C)
    )
    init_fv = _grid_sample_3d(fm, xyz[..., ::-1], "bilinear", fmt=fmt_fm)  # (N,K,C)
    ncrd = _find_neighbor_coords(xyz, fm.shape)  # (N,K,A,3)
    A = ncrd.shape[2]
    grid_n = ncrd.reshape(N, K * A, 3)[..., ::-1]
    lin_n, valid_n = _nearest_lin(grid_n, D, H, W)  # shared by nf and sf
    nf = _fetch_lin(fmt_fm, lin_n, valid_n)
    rd = np.linalg.norm(
        xyz[:, :, None, None, :] - ncrd[:, :, None, :, :], axis=-1
    ).astype(np.float32)
    rw = 1.0 / (rd + np.float32(1e-6))
    rw = (rw / rw.sum(-1, keepdims=True)).reshape(N, K, 1, A)  # (N,K,1,A)

    th.join()
    idx = dev.get("idx")
    if idx is None:
        idx = _host_topk(sl_full)

    # ---- index-weighted neighbor combine (host) ----
    featsT = np.ascontiguousarray(np.transpose(sl_full, (0, 1, 3, 2))).reshape(
        S * N, L, C
    )
    dist = np.abs(idx - np.arange(L)[None, None, :, None]).astype(
        np.float32
    ) + np.float32(1e-5)
    w = 1.0 / dist
    w = (w / w.sum(-1, keepdims=True)).astype(np.float32).reshape(S * N, L, TOP_K)
    idx_f = idx.reshape(S * N, L, TOP_K)
    g5 = np.take_along_axis(
        featsT, idx_f.reshape(S * N, L * TOP_K, 1), axis=1
    ).reshape(S * N, L, TOP_K, C)
    wa_lc = (w[:, :, None, :] @ g5).reshape(S * N, L, C)
    # Direct permutation of wa_lc (S,N,L,C) to the voxel-major layout the
    # nearest-sample needs — equivalent to building sim_feats=(N,C,D,H,W) and
    # re-transposing, but with one copy instead of two. Index algebra:
    # sim_feats[n,c,d,h,w] = wa[4n + c//32, (c%32)//8, (c%8)*16 + d, h*32+w].
    sim_fmt = np.ascontiguousarray(
        wa_lc.reshape(4, 4, 4, L, 8, 16).transpose(0, 5, 3, 1, 2, 4)
    ).reshape(N, D * H * W, C)

    sf = _fetch_lin(sim_fmt, lin_n, valid_n)
    # comb = ((nf_v*rw).sum(2)+(sf_v*rw).sum(2))/2 == ((nf_v+sf_v)*rw).sum(2)/2,
    # so add before the raw (N,C,P)->(N,K,A,C) view and weight once. The
    # torch view maps (k,a,c) -> s[n, (k%64)*A*C + a*C + c, k//64] (since
    # P = K*A = 64*A*C here), so contract from that strided view directly
    # instead of materialising the 300MB (N,C,P) transpose copy.
    sr = (nf + sf).reshape(N, 64, A, C, 128)  # [n, k%64, a, c, k//64]
    rwr = rw.reshape(N, 128, 64, A)  # [n, k//64, k%64, a]
    comb = np.einsum("nqma,nmafq->nqmf", rwr, sr, optimize=True).reshape(
        N, K, C
    ) / np.float32(2.0)

    # ---- projections + 4-token attention (seq axis = N, batch = K) ----
    q = init_fv @ Wq.T + bq
    k = comb @ Wk.T + bk
    v = comb @ Wv.T + bv
    E = C
    hd = E // NUM_HEADS
    qp = (q @ ipw[:E].T + ipb[:E]).reshape(N, K, NUM_HEADS, hd)
    kp = (k @ ipw[E : 2 * E].T + ipb[E : 2 * E]).reshape(N, K, NUM_HEADS, hd)
    vp = (v @ ipw[2 * E :].T + ipb[2 * E :]).reshape(N, K, NUM_HEADS, hd)
    qb = np.ascontiguousarray(np.transpose(qp, (1, 2, 0, 3)))  # (K,H,N,hd)
    kb = np.ascontiguousarray(np.transpose(kp, (1, 2, 3, 0)))  # (K,H,hd,M)
    vb = np.ascontiguousarray(np.transpose(vp, (1, 2, 0, 3)))  # (K,H,M,hd)
    scores = (qb @ kb) / np.float32(np.sqrt(hd))  # (K,H,N,M)
    scores = scores - scores.max(-1, keepdims=True)
    e = np.exp(scores)
    attn = e / e.sum(-1, keepdims=True)
    ao = np.ascontiguousarray(
        np.transpose(attn @ vb, (2, 0, 1, 3))  # (N,K,H,hd)
    ).reshape(N, K, E)
    ao = ao @ ow.T + ob
    return (ao + init_fv).astype(np.float32)


# revision 15
# speedup vs baseline: 3.1213x; 1.2673x over previous
"""AttentionGuidedInterpolation kernel for 8 Trainium2 NeuronCores.

Device (Bass/Tile, SPMD x8): the compute-heavy similarity search —
64 gram matrices (128-dim features, 1024x1024 each, 17.2 GFLOP) on the
TensorEngine, PSUM->SBUF bf16 downcast on the Activation engine, and
top-8 row search via the DVE Max8/MaxIndex instructions. Each core
handles 8 of the 64 independent (slice, batch) units.

The per-core schedule is DVE-roofline-bound: CoreSim shows the DVE at
94.6% occupancy with zero gaps (Max + MaxIndex are mandatory full-row
scans at 1 elem/lane/cycle; neither supports the 2x packed perf modes,
there is no fused max-with-index instruction, and no other engine can
pre-reduce a row for it). PE sits at 18%, Act at 52% — the algorithmic
floor for exact per-row top-k on this hardware.

Dispatch: run_bass_kernel_spmd under axon re-builds and re-jits its
shard_map wrapper on every call (new closure -> jit cache miss -> full
XLA retrace + compile + neuronx cache lookup, several hundred ms of
pure host/RPC overhead per call). We inline the same bass2jax lowering
it uses (_bass_exec_p custom call on the 8-device mesh) but cache the
jitted executable, pre-stage inputs asynchronously, and create the
donated output buffers on-device, so the per-call cost is one blocking
execute round-trip. Falls back to run_bass_kernel_spmd, then to a
numpy top-k, if the internal API is unavailable.

Host (numpy): index-weighted neighbor combine, grid samples, and the
tiny 4-token attention — cheap glue driven by the device-computed
indices, overlapped with the device call.
"""

import sys
import threading
import time

if "/opt/trn_rl_repo" not in sys.path:
    sys.path.insert(0, "/opt/trn_rl_repo")

import numpy as np

TOP_K = 5
R = 1
NUM_HEADS = 8
N, C, D, H, W, K = 4, 128, 16, 32, 32, 8192
S, L = D, H * W  # 16 slices, 1024 positions per slice
N_CORES = 8
UPC = (S * N) // N_CORES  # units per core = 8

_cache = {}


ENC_M = float(3 * 2**22)  # magic round-to-integer constant (ulp = 1)

# Device algorithm. "hybrid" runs 5 of each unit's 8 row-tiles on the
# exact 2-scan path (Max + MaxIndex) and 3 (ENC_LTS) on the single-scan
# encoded path: the scan path is DVE-bound with PE/Act mostly idle, the
# enc path is PE/Act-bound with DVE half-idle, so mixing them trims
# total DVE work and fills the DVE dependency bubbles between each scan
# tile's max -> max_index pair. The enc tiles are emitted LAST per unit
# in stage order (all grams, all Act rounds, all -M adds, all frac
# adds) so the 1-row accumulate matmuls with identical weights run
# back-to-back — PE weight-swap stalls made per-tile emission lose.
# Three enc tiles, not four: the fourth would pin all 16KB of PSUM and
# serialize against the next unit's scan tiles. Measured head-to-head
# in shared tunnel-drift windows: ~41 us vs 68 (2 enc tiles,
# per-tile emission) vs ~118 (all-scan) vs ~109 (all-enc). "scan"
# remains the fallback if the enc tiles' PSUM encode round-trip
# misbehaves (decode sanity check).
ALGO = "hybrid"
ENC_LTS = (5, 6, 7)


def _build_bass(reps=1, algo="enc"):
    """Build the gram + top-k program.

    algo="enc" (default): single-DVE-scan encoded top-8. The Act engine
    rewrites each PSUM gram bank in place as fl(16*sim + M) — the f32
    add against M = 3*2^22 (ulp 1) rounds the similarity to an integer
    grid (quantum 1/16, finer than the bf16 ranking it replaces). Two
    cheap 1-row accumulate matmuls then add -M (exact by Sterbenz) and
    (1023-j)/1024 (exact: |Q| <= 2^13 leaves 10 mantissa bits for the
    fraction; every k/1024 is exact in fp16). One DVE max8 over the
    encoded PSUM returns the top-8 values with their column indices
    embedded in the fraction — no max_index scan. This halves the DVE
    work that bounds the 2-scan variant.

    algo="scan": the classic exact path (bf16 downcast + DVE Max +
    MaxIndex) — the runtime fallback if the encoded path's PSUM
    accumulate semantics differ on silicon (detected by the decode
    sanity check in _run_device_topk).

    reps > 1 unrolls the identical per-core workload `reps` times (only
    rep 0 stores outputs) — used by test.py to measure steady-state
    per-execution HW time differentially, cancelling the axon dispatch
    round-trip that dwarfs a single execution.
    """
    import concourse.mybir as mybir
    from concourse import bacc, tile
    from concourse._compat import get_trn_type

    f32 = mybir.dt.float32
    bf16 = mybir.dt.bfloat16
    f16 = mybir.dt.float16
    u16 = mybir.dt.uint16

    nc = bacc.Bacc(
        get_trn_type(),
        target_bir_lowering=False,
        debug=False,
        num_devices=N_CORES,
    )
    sl_in = nc.dram_tensor("sl", [UPC, 128, L], bf16, kind="ExternalInput")

    if algo == "hybrid":
        negM_in = nc.dram_tensor("negM", [1, L], bf16, kind="ExternalInput")
        frac_in = nc.dram_tensor("frac", [1, L], f16, kind="ExternalInput")
        idxs_out = nc.dram_tensor("idxs", [UPC, L, TOP_K], u16, kind="ExternalOutput")
        enc_out = nc.dram_tensor("enc", [UPC, L, TOP_K], f32, kind="ExternalOutput")
        with tile.TileContext(nc) as tc:
            with (
                tc.tile_pool(name="sb", bufs=3) as pool,
                tc.tile_pool(name="cstp", bufs=8) as cstpool,
                tc.tile_pool(name="simp", bufs=4) as simpool,
                tc.tile_pool(name="mxp", bufs=8) as mxpool,
                tc.tile_pool(name="ps", bufs=4, space="PSUM") as pp,
            ):
                negM = cstpool.tile([1, L], bf16, tag="negM")
                nc.sync.dma_start(out=negM[:], in_=negM_in[:])
                frac = cstpool.tile([1, L], f16, tag="frac")
                nc.sync.dma_start(out=frac[:], in_=frac_in[:])
                ones_bf = cstpool.tile([1, 128], bf16, tag="onesb")
                nc.vector.memset(ones_bf[:], 1.0)
                ones_f16 = cstpool.tile([1, 128], f16, tag="onesh")
                nc.vector.memset(ones_f16[:], 1.0)
                biasM = cstpool.tile([128, 1], f32, tag="biasM")
                nc.vector.memset(biasM[:], ENC_M)
                for rep in range(reps):
                    for u in range(UPC):
                        sl_t = pool.tile([128, L], bf16, tag="sl")
                        nc.sync.dma_start(out=sl_t[:], in_=sl_in[u])
                        # scan tiles first (release their PSUM quickly)...
                        for lt in range(L // 128):
                            if lt in ENC_LTS:
                                continue
                            ps = pp.tile([128, L], f32, tag="ps")
                            lhsT = sl_t[:, lt * 128 : (lt + 1) * 128]
                            nc.tensor.matmul(ps[:, 0:512], lhsT, sl_t[:, 0:512])
                            nc.tensor.matmul(ps[:, 512:1024], lhsT, sl_t[:, 512:1024])
                            sim_bf = simpool.tile([128, L], bf16, tag="sim")
                            nc.scalar.copy(sim_bf[:, 0:512], ps[:, 0:512])
                            nc.scalar.copy(sim_bf[:, 512:1024], ps[:, 512:1024])
                            mxb = mxpool.tile([128, 8], bf16, tag="mxb")
                            ix = mxpool.tile([128, 8], u16, tag="ix")
                            nc.vector.max(mxb[:], sim_bf[:])
                            nc.vector.max_index(ix[:], mxb[:], sim_bf[:])
                            if rep == 0:
                                nc.sync.dma_start(
                                    out=idxs_out[u, lt * 128 : (lt + 1) * 128, :],
                                    in_=ix[:, 0:TOP_K],
                                )
                        # ...then the enc tiles in stage order, so the 1-row
                        # matmuls with identical weights run back-to-back.
                        etiles = {}
                        for lt in ENC_LTS:
                            ps = pp.tile([128, L], f32, tag="ps")
                            lhsT = sl_t[:, lt * 128 : (lt + 1) * 128]
                            for h in (0, 1):
                                nc.tensor.matmul(
                                    ps[:, h * 512 : (h + 1) * 512],
                                    lhsT,
                                    sl_t[:, h * 512 : (h + 1) * 512],
                                )
                            etiles[lt] = ps
                        for lt in ENC_LTS:
                            ps = etiles[lt]
                            for h in (0, 1):
                                nc.scalar.activation(
                                    ps[:, h * 512 : (h + 1) * 512],
                                    ps[:, h * 512 : (h + 1) * 512],
                                    mybir.ActivationFunctionType.Identity,
                                    bias=biasM[:],
                                    scale=16.0,
                                )
                        for lt in ENC_LTS:
                            ps = etiles[lt]
                            for h in (0, 1):
                                nc.tensor.matmul(
                                    ps[:, h * 512 : (h + 1) * 512],
                                    ones_bf[:],
                                    negM[:, h * 512 : (h + 1) * 512],
                                    start=False,
                                    stop=False,
                                    skip_group_check=True,
                                )
                        for lt in ENC_LTS:
                            ps = etiles[lt]
                            for h in (0, 1):
                                nc.tensor.matmul(
                                    ps[:, h * 512 : (h + 1) * 512],
                                    ones_f16[:],
                                    frac[:, h * 512 : (h + 1) * 512],
                                    start=False,
                                    stop=True,
                                    skip_group_check=True,
                                )
                        for lt in ENC_LTS:
                            mx = mxpool.tile([128, 8], f32, tag="mxf")
                            nc.vector.max(mx[:], etiles[lt][:])
                            if rep == 0:
                                nc.sync.dma_start(
                                    out=enc_out[u, lt * 128 : (lt + 1) * 128, :],
                                    in_=mx[:, 0:TOP_K],
                                )
        nc.compile()
        return nc

    if algo == "enc":
        negM_in = nc.dram_tensor("negM", [1, L], bf16, kind="ExternalInput")
        frac_in = nc.dram_tensor("frac", [1, L], f16, kind="ExternalInput")
        enc_out = nc.dram_tensor("enc", [UPC, L, TOP_K], f32, kind="ExternalOutput")
        with tile.TileContext(nc) as tc:
            with (
                tc.tile_pool(name="sb", bufs=3) as pool,
                tc.tile_pool(name="cstp", bufs=6) as cstpool,
                tc.tile_pool(name="mxp", bufs=8) as mxpool,
                tc.tile_pool(name="ps", bufs=4, space="PSUM") as pp,
            ):
                negM = cstpool.tile([1, L], bf16, tag="negM")
                nc.sync.dma_start(out=negM[:], in_=negM_in[:])
                frac = cstpool.tile([1, L], f16, tag="frac")
                nc.sync.dma_start(out=frac[:], in_=frac_in[:])
                ones_bf = cstpool.tile([1, 128], bf16, tag="onesb")
                nc.vector.memset(ones_bf[:], 1.0)
                ones_f16 = cstpool.tile([1, 128], f16, tag="onesh")
                nc.vector.memset(ones_f16[:], 1.0)
                biasM = cstpool.tile([128, 1], f32, tag="biasM")
                nc.vector.memset(biasM[:], ENC_M)
                for rep in range(reps):
                    for u in range(UPC):
                        sl_t = pool.tile([128, L], bf16, tag="sl")
                        nc.sync.dma_start(out=sl_t[:], in_=sl_in[u])
                        for lt in range(L // 128):
                            ps = pp.tile([128, L], f32, tag="ps")
                            lhsT = sl_t[:, lt * 128 : (lt + 1) * 128]
                            for h in (0, 1):
                                sl_h = sl_t[:, h * 512 : (h + 1) * 512]
                                p = ps[:, h * 512 : (h + 1) * 512]
                                nc.tensor.matmul(p, lhsT, sl_h)
                                nc.scalar.activation(
                                    p,
                                    p,
                                    mybir.ActivationFunctionType.Identity,
                                    bias=biasM[:],
                                    scale=16.0,
                                )
                                nc.tensor.matmul(
                                    p,
                                    ones_bf[:],
                                    negM[:, h * 512 : (h + 1) * 512],
                                    start=False,
                                    stop=False,
                                    skip_group_check=True,
                                )
                                nc.tensor.matmul(
                                    p,
                                    ones_f16[:],
                                    frac[:, h * 512 : (h + 1) * 512],
                                    start=False,
                                    stop=True,
                                    skip_group_check=True,
                                )
                            mx = mxpool.tile([128, 8], f32, tag="mx")
                            nc.vector.max(mx[:], ps[:])
                            if rep == 0:
                                nc.sync.dma_start(
                                    out=enc_out[u, lt * 128 : (lt + 1) * 128, :],
                                    in_=mx[:, 0:TOP_K],
                                )
        nc.compile()
        return nc

    idxs_out = nc.dram_tensor("idxs", [UPC, L, TOP_K], u16, kind="ExternalOutput")
    with tile.TileContext(nc) as tc:
        with (
            tc.tile_pool(name="sb", bufs=3) as pool,
            tc.tile_pool(name="mxp", bufs=8) as mxpool,
            tc.tile_pool(name="simp", bufs=4) as simpool,
            tc.tile_pool(name="ps", bufs=3, space="PSUM") as pp,
        ):
            for rep in range(reps):
                for u in range(UPC):
                    sl_t = pool.tile([128, L], bf16, tag="sl")
                    nc.sync.dma_start(out=sl_t[:], in_=sl_in[u])
                    for lt in range(L // 128):
                        ps = pp.tile([128, L], f32, tag="ps")
                        lhsT = sl_t[:, lt * 128 : (lt + 1) * 128]
                        # two matmuls: a PSUM bank holds 512 fp32 per partition
                        nc.tensor.matmul(ps[:, 0:512], lhsT, sl_t[:, 0:512])
                        nc.tensor.matmul(ps[:, 512:1024], lhsT, sl_t[:, 512:1024])
                        # bf16 ranking copy: sim values only rank neighbors;
                        # the result is insensitive to rank jitter beyond the
                        # (always exact) self-match because its 1/1e-5 weight
                        # dominates the index-distance softmax.
                        sim_bf = simpool.tile([128, L], bf16, tag="sim")
                        nc.scalar.copy(sim_bf[:, 0:512], ps[:, 0:512])
                        nc.scalar.copy(sim_bf[:, 512:1024], ps[:, 512:1024])
                        mx = mxpool.tile([128, 8], bf16, tag="mx")
                        ix = mxpool.tile([128, 8], u16, tag="ix")
                        nc.vector.max(mx[:], sim_bf[:])
                        nc.vector.max_index(ix[:], mx[:], sim_bf[:])
                        if rep == 0:
                            nc.sync.dma_start(
                                out=idxs_out[u, lt * 128 : (lt + 1) * 128, :],
                                in_=ix[:, 0:TOP_K],
                            )
    nc.compile()
    return nc


class _CachedRunner:
    """bass2jax axon dispatch with a cached jitted executable.

    Mirrors what run_bass_kernel_spmd does under axon (the _bass_exec_p
    custom call inside a shard_map over the 8-core mesh) but builds the
    jit exactly once, so repeat calls skip the retrace/compile.
    """

    def __init__(self, nc):
        import jax
        import jax.numpy as jnp
        from jax.sharding import Mesh, NamedSharding, PartitionSpec

        try:
            from jax.experimental.shard_map import shard_map
        except ImportError:  # newer jax
            from jax import shard_map

        import concourse.mybir as mybir
        from concourse.bass2jax import (
            _bass_exec_p,
            install_neuronx_cc_hook,
            partition_id_tensor,
        )

        install_neuronx_cc_hook()
        self.jax = jax
        self.nc = nc

        partition_name = (
            nc.partition_id_tensor.name if nc.partition_id_tensor else None
        )
        in_names, out_names, out_avals, zero_shapes = [], [], [], []
        for alloc in nc.m.functions[0].allocations:
            if not isinstance(alloc, mybir.MemoryLocationSet):
                continue
            name = alloc.memorylocations[0].name
            if alloc.kind == "ExternalInput":
                if name != partition_name:
                    in_names.append(name)
            elif alloc.kind == "ExternalOutput":
                shape = tuple(alloc.tensor_shape)
                dtype = mybir.dt.np(alloc.dtype)
                out_names.append(name)
                out_avals.append(jax.core.ShapedArray(shape, dtype))
                zero_shapes.append((shape, dtype))
        n_params, n_outs = len(in_names), len(out_avals)
        self.in_names = in_names
        self.out_names = out_names
        in_names_all = in_names + out_names
        if partition_name is not None:
            in_names_all.append(partition_name)

        def _body(*args):
            ops = list(args)
            if partition_name is not None:
                ops.append(partition_id_tensor())
            return tuple(
                _bass_exec_p.bind(
                    *ops,
                    out_avals=tuple(out_avals),
                    in_names=tuple(in_names_all),
                    out_names=tuple(out_names),
                    lowering_input_output_aliases=(),
                    sim_require_finite=True,
                    sim_require_nnan=True,
                    nc=nc,
                )
            )

        devices = jax.devices()[:N_CORES]
        assert len(devices) == N_CORES
        mesh = Mesh(np.asarray(devices), ("core",))
        self.spec = NamedSharding(mesh, PartitionSpec("core"))
        self.sharded = jax.jit(
            shard_map(
                _body,
                mesh=mesh,
                in_specs=(PartitionSpec("core"),) * (n_params + n_outs),
                out_specs=(PartitionSpec("core"),) * n_outs,
                check_rep=False,
            ),
            donate_argnums=tuple(range(n_params, n_params + n_outs)),
            keep_unused=True,
        )
        # Donated output buffers, created on-device (no H2D bytes).
        self.mkzeros = jax.jit(
            lambda: tuple(
                jnp.zeros((N_CORES * s[0], *s[1:]), d) for s, d in zero_shapes
            ),
            out_shardings=(self.spec,) * n_outs,
        )

    def stage(self, in_map):
        """Async H2D of global (cores-concatenated) inputs + donated outputs.

        in_map: {name: global array with axis 0 = n_cores * per_core_dim0}.
        """
        dins = [self.jax.device_put(in_map[n], self.spec) for n in self.in_names]
        zs = self.mkzeros()
        return dins, zs

    def execute(self, dins, zs, block=True):
        """Dispatch the NEFF; returns (device outputs, blocking span ns)."""
        t0 = time.perf_counter()
        out = self.sharded(*dins, *zs)
        if block:
            self.jax.block_until_ready(out)
        span = (time.perf_counter() - t0) * 1e9
        return out, span

    def fetch(self, out):
        """D2H with all shard transfers in flight before any blocks."""
        arrs = []
        for o in out:
            shards = o.addressable_shards
            for s_ in shards:
                s_.data.copy_to_host_async()
            arrs.append(
                np.concatenate([np.asarray(s_.data) for s_ in shards], axis=0)
            )
        return dict(zip(self.out_names, arrs))


def _get_runner(reps=1, algo="enc"):
    key = ("runner", reps, algo)
    if key not in _cache:
        _cache[key] = _CachedRunner(_build_bass(reps=reps, algo=algo))
    return _cache[key]


def _enc_consts():
    """Per-core const rows for the encoded path, replicated across cores."""
    import ml_dtypes

    negM = np.full((N_CORES, L), -ENC_M, np.float32).astype(ml_dtypes.bfloat16)
    frac = np.tile(
        ((1023.0 - np.arange(L)) / 1024.0).astype(np.float16)[None, :], (N_CORES, 1)
    )
    return negM, frac


def _decode_enc(enc):
    """enc (64, L, 5) f32 -> idx int64 + sanity flag.

    enc = Q + (1023-j)/1024 with Q = round(16*sim) an integer: exact in
    f32 for |Q| < 2^13, so the decode recovers j exactly. Sanity: the
    top-1 of a gram row is its own diagonal (self-similarity dominates
    by ~8 quantization sigmas); if the PSUM round-trip behaved
    differently on silicon the fractions collapse and this check fails.
    """
    Q = np.floor(enc)
    j = 1023 - np.rint((enc - Q) * 1024.0).astype(np.int64)
    ok = bool((j >= 0).all() and (j <= 1023).all())
    if ok:
        diag = np.arange(L)[None, :]
        ok = float(np.mean(j[:, :, 0] == diag)) > 0.999
    return j, ok


def _host_topk(sl_full):
    """Numpy fallback: exact gram + top-5 (jax tie-break: value desc, index asc)."""
    slb = sl_full.reshape(S * N, C, L)
    sim = np.matmul(np.transpose(slb, (0, 2, 1)), slb).reshape(S, N, L, L)
    part = np.argpartition(-sim, TOP_K, axis=-1)[..., :TOP_K]
    pvals = np.take_along_axis(sim, part, axis=-1)
    order = np.lexsort((part, -pvals), axis=-1)
    idx = np.take_along_axis(part, order, axis=-1)
    return idx.astype(np.int64)  # (S,N,L,5)


def _to_bf16_units(sl_full):
    import ml_dtypes

    return np.ascontiguousarray(sl_full.reshape(S * N, C, L)).astype(
        ml_dtypes.bfloat16
    )


def _run_device_topk(sl_full):
    """sl_full: (S, N, C, L) f32. Returns idx (S,N,L,5) int64 via 8 cores."""
    sl_units = _to_bf16_units(sl_full)
    t0 = time.perf_counter()
    idx = None
    if ALGO == "hybrid" and not _cache.get("enc_bad"):
        try:  # mixed scan/enc tiles: merge the two output tensors
            runner = _get_runner(reps=1, algo="hybrid")
            negM, frac = _enc_consts()
            dins, zs = runner.stage({"sl": sl_units, "negM": negM, "frac": frac})
            out, span = runner.execute(dins, zs)
            res = runner.fetch(out)
            _cache["exec_span_ns"] = span
            merged = res["idxs"].reshape(S * N, L, TOP_K).astype(np.int64)
            enc = res["enc"].reshape(S * N, L, TOP_K)
            for lt in ENC_LTS:
                sl_rows = slice(lt * 128, (lt + 1) * 128)
                e = enc[:, sl_rows]
                Q = np.floor(e)
                merged[:, sl_rows] = 1023 - np.rint((e - Q) * 1024.0).astype(
                    np.int64
                )
            ok = bool((merged >= 0).all() and (merged <= 1023).all())
            if ok:
                diag = np.arange(L)[None, :]
                ok = float(np.mean(merged[:, :, 0] == diag)) > 0.999
            if ok:
                idx = merged
            else:
                _cache["enc_bad"] = True
        except Exception:  # pragma: no cover - harness-proofing
            _cache["enc_bad"] = True
    if ALGO == "enc" and not _cache.get("enc_bad"):
        try:  # encoded single-scan path
            runner = _get_runner(reps=1, algo="enc")
            negM, frac = _enc_consts()
            dins, zs = runner.stage({"sl": sl_units, "negM": negM, "frac": frac})
            out, span = runner.execute(dins, zs)
            res = runner.fetch(out)
            _cache["exec_span_ns"] = span
            j, ok = _decode_enc(res["enc"].reshape(S * N, L, TOP_K))
            if ok:
                idx = j
            else:  # silicon disagreed with the PSUM encode round-trip
                _cache["enc_bad"] = True
        except Exception:  # pragma: no cover - harness-proofing
            _cache["enc_bad"] = True
    if idx is None:
        try:  # exact 2-scan DVE path
            runner = _get_runner(reps=1, algo="scan")
            dins, zs = runner.stage({"sl": sl_units})
            out, span = runner.execute(dins, zs)
            res = runner.fetch(out)
            _cache["exec_span_ns"] = span
            idx = res["idxs"]
        except Exception:  # pragma: no cover
            from concourse.bass_utils import run_bass_kernel_spmd

            if "nc" not in _cache:
                _cache["nc"] = _build_bass(reps=1, algo="scan")
            in_maps = [
                {"sl": np.ascontiguousarray(sl_units[c * UPC : (c + 1) * UPC])}
                for c in range(N_CORES)
            ]
            out = run_bass_kernel_spmd(_cache["nc"], in_maps, list(range(N_CORES)))
            idx = np.concatenate(
                [np.asarray(out.results[c]["idxs"]) for c in range(N_CORES)], 0
            )
    _cache["last_device_ns"] = (time.perf_counter() - t0) * 1e9
    idx = np.clip(idx.reshape(S, N, L, TOP_K).astype(np.int64), 0, L - 1)
    return idx


# ---------------- numpy ports of the reference glue ----------------


def _unnorm(g, size):
    return ((g + 1.0) * size - 1.0) / 2.0


def _grid_sample_3d(fm, grid, mode, fmt=None):
    # fm: (N,C,Dd,Hh,Ww); grid: (N,P,3) last dim (x->W, y->H, z->D)
    # fmt: optional precomputed voxel-major view (N, D*H*W, C)
    n_, c_, d_, h_, w_ = fm.shape
    if fmt is None:
        fmt = np.ascontiguousarray(
            np.transpose(fm, (0, 2, 3, 4, 1)).reshape(n_, d_ * h_ * w_, c_)
        )
    ix = _unnorm(grid[..., 0], w_)
    iy = _unnorm(grid[..., 1], h_)
    iz = _unnorm(grid[..., 2], d_)

    def fetch(z, y, x):
        valid = (z >= 0) & (z < d_) & (y >= 0) & (y < h_) & (x >= 0) & (x < w_)
        lin = (
            np.clip(z, 0, d_ - 1) * (h_ * w_)
            + np.clip(y, 0, h_ - 1) * w_
            + np.clip(x, 0, w_ - 1)
        )
        v = np.take_along_axis(fmt, lin[..., None], axis=1)
        v[~valid] = 0.0
        return v

    if mode == "nearest":
        return fetch(
            np.round(iz).astype(np.int64),
            np.round(iy).astype(np.int64),
            np.round(ix).astype(np.int64),
        )
    x0 = np.floor(ix)
    y0 = np.floor(iy)
    z0 = np.floor(iz)
    tx, ty, tz = ix - x0, iy - y0, iz - z0
    x0i, y0i, z0i = x0.astype(np.int64), y0.astype(np.int64), z0.astype(np.int64)
    out = np.zeros(grid.shape[:-1] + (c_,), fm.dtype)
    for dz in (0, 1):
        for dy in (0, 1):
            for dx in (0, 1):
                wgt = (
                    (tz if dz else 1.0 - tz)
                    * (ty if dy else 1.0 - ty)
                    * (tx if dx else 1.0 - tx)
                ).astype(np.float32)
                out += fetch(z0i + dz, y0i + dy, x0i + dx) * wgt[..., None]
    return out  # (N,P,C)


def _nearest_lin(grid, d_, h_, w_):
    """Shared nearest-voxel linear indices + validity for a (N,P,3) grid."""
    ix = _unnorm(grid[..., 0], w_)
    iy = _unnorm(grid[..., 1], h_)
    iz = _unnorm(grid[..., 2], d_)
    z = np.round(iz).astype(np.int64)
    y = np.round(iy).astype(np.int64)
    x = np.round(ix).astype(np.int64)
    valid = (z >= 0) & (z < d_) & (y >= 0) & (y < h_) & (x >= 0) & (x < w_)
    lin = (
        np.clip(z, 0, d_ - 1) * (h_ * w_)
        + np.clip(y, 0, h_ - 1) * w_
        + np.clip(x, 0, w_ - 1)
    )
    return lin, valid


def _fetch_lin(fmt, lin, valid):
    v = np.take_along_axis(fmt, lin[..., None], axis=1)
    v[~valid] = 0.0
    return v


def _find_neighbor_coords(xyz_hr, fm_shape, r=R):
    d_, h_, w_ = fm_shape[-3:]
    scale = np.array([d_ - 1, h_ - 1, w_ - 1], np.float32)
    g = np.floor((xyz_hr + 1.0) / 2.0 * scale).astype(np.float32)
    steps = np.linspace(-float(r), float(r), 2 * r + 1).astype(np.float32)
    dh, dv = steps * np.float32(2.0 / h_), steps * np.float32(2.0 / w_)
    # mdi == 0 for these shapes (D=16 smallest)
    d2 = np.stack(np.meshgrid(dh, dv, indexing="ij"), -1).reshape(1, 1, -1, 2)
    nc2 = g[..., 1:][:, :, None, :] + d2
    fixed = np.broadcast_to(g[..., 0:1][:, :, None, :], nc2.shape[:3] + (1,))
    ncrd = np.concatenate([fixed, nc2], -1).astype(np.float32)
    return ncrd / scale * 2.0 - 1.0  # (N,K,A,3)


def kernel(**inputs):
    fm = np.asarray(inputs["feature_map"], np.float32)
    xyz = np.asarray(inputs["xyz_hr"], np.float32)
    Wq = np.asarray(inputs["Wq"], np.float32)
    bq = np.asarray(inputs["bq"], np.float32)
    Wk = np.asarray(inputs["Wk"], np.float32)
    bk = np.asarray(inputs["bk"], np.float32)
    Wv = np.asarray(inputs["Wv"], np.float32)
    bv = np.asarray(inputs["bv"], np.float32)
    ipw = np.asarray(inputs["in_proj_w"], np.float32)
    ipb = np.asarray(inputs["in_proj_b"], np.float32)
    ow = np.asarray(inputs["out_w"], np.float32)
    ob = np.asarray(inputs["out_b"], np.float32)

    # ---- similarity search: gram + top-8 on the 8 NeuronCores ----
    sl_full = np.ascontiguousarray(
        np.transpose(fm, (2, 0, 1, 3, 4)).reshape(S, N, C, L)
    )

    # Run the device top-k concurrently with the host-side sampling work
    # that does not depend on it (bilinear init_fv, neighbor coords, nf).
    dev = {}

    def _dev_worker():
        try:
            dev["idx"] = _run_device_topk(sl_full)  # (S,N,L,5)
        except Exception as e:  # device path unavailable -> host fallback
            dev["err"] = e

    th = threading.Thread(target=_dev_worker)
    th.start()

    # ---- device-independent sampling work (overlapped with the device call) ----
    fmt_fm = np.ascontiguousarray(
        np.transpose(fm, (0, 2, 3, 4, 1)).reshape(N, D * H * W, C)
    )
    init_fv = _grid_sample_3d(fm, xyz[..., ::-1], "bilinear", fmt=fmt_fm)  # (N,K,C)
    ncrd = _find_neighbor_coords(xyz, fm.shape)  # (N,K,A,3)
    A = ncrd.shape[2]
    grid_n = ncrd.reshape(N, K * A, 3)[..., ::-1]
    lin_n, valid_n = _nearest_lin(grid_n, D, H, W)  # shared by nf and sf
    nf = _fetch_lin(fmt_fm, lin_n, valid_n)
    rd = np.linalg.norm(
        xyz[:, :, None, None, :] - ncrd[:, :, None, :, :], axis=-1
    ).astype(np.float32)
    rw = 1.0 / (rd + np.float32(1e-6))
    rw = (rw / rw.sum(-1, keepdims=True)).reshape(N, K, 1, A)  # (N,K,1,A)

    th.join()
    idx = dev.get("idx")
    if idx is None:
        idx = _host_topk(sl_full)

    # ---- index-weighted neighbor combine (host) ----
    featsT = np.ascontiguousarray(np.transpose(sl_full, (0, 1, 3, 2))).reshape(
        S * N, L, C
    )
    dist = np.abs(idx - np.arange(L)[None, None, :, None]).astype(
        np.float32
    ) + np.float32(1e-5)
    w = 1.0 / dist
    w = (w / w.sum(-1, keepdims=True)).astype(np.float32).reshape(S * N, L, TOP_K)
    idx_f = idx.reshape(S * N, L, TOP_K)
    g5 = np.take_along_axis(
        featsT, idx_f.reshape(S * N, L * TOP_K, 1), axis=1
    ).reshape(S * N, L, TOP_K, C)
    wa_lc = (w[:, :, None, :] @ g5).reshape(S * N, L, C)
    # Direct permutation of wa_lc (S,N,L,C) to the voxel-major layout the
    # nearest-sample needs — equivalent to building sim_feats=(N,C,D,H,W) and
    # re-transposing, but with one copy instead of two. Index algebra:
    # sim_feats[n,c,d,h,w] = wa[4n + c//32, (c%32)//8, (c%8)*16 + d, h*32+w].
    sim_fmt = np.ascontiguousarray(
        wa_lc.reshape(4, 4, 4, L, 8, 16).transpose(0, 5, 3, 1, 2, 4)
    ).reshape(N, D * H * W, C)

    sf = _fetch_lin(sim_fmt, lin_n, valid_n)
    # comb = ((nf_v*rw).sum(2)+(sf_v*rw).sum(2))/2 == ((nf_v+sf_v)*rw).sum(2)/2,
    # so add before the raw (N,C,P)->(N,K,A,C) view and weight once. The
    # torch view maps (k,a,c) -> s[n, (k%64)*A*C + a*C + c, k//64] (since
    # P = K*A = 64*A*C here), so contract from that strided view directly
    # instead of materialising the 300MB (N,C,P) transpose copy.
    sr = (nf + sf).reshape(N, 64, A, C, 128)  # [n, k%64, a, c, k//64]
    rwr = rw.reshape(N, 128, 64, A)  # [n, k//64, k%64, a]
    comb = np.einsum("nqma,nmafq->nqmf", rwr, sr, optimize=True).reshape(
        N, K, C
    ) / np.float32(2.0)

    # ---- projections + 4-token attention (seq axis = N, batch = K) ----
    q = init_fv @ Wq.T + bq
    k = comb @ Wk.T + bk
    v = comb @ Wv.T + bv
    E = C
    hd = E // NUM_HEADS
    qp = (q @ ipw[:E].T + ipb[:E]).reshape(N, K, NUM_HEADS, hd)
    kp = (k @ ipw[E : 2 * E].T + ipb[E : 2 * E]).reshape(N, K, NUM_HEADS, hd)
    vp = (v @ ipw[2 * E :].T + ipb[2 * E :]).reshape(N, K, NUM_HEADS, hd)
    qb = np.ascontiguousarray(np.transpose(qp, (1, 2, 0, 3)))  # (K,H,N,hd)
    kb = np.ascontiguousarray(np.transpose(kp, (1, 2, 3, 0)))  # (K,H,hd,M)
    vb = np.ascontiguousarray(np.transpose(vp, (1, 2, 0, 3)))  # (K,H,M,hd)
    scores = (qb @ kb) / np.float32(np.sqrt(hd))  # (K,H,N,M)
    scores = scores - scores.max(-1, keepdims=True)
    e = np.exp(scores)
    attn = e / e.sum(-1, keepdims=True)
    ao = np.ascontiguousarray(
        np.transpose(attn @ vb, (2, 0, 1, 3))  # (N,K,H,hd)
    ).reshape(N, K, E)
    ao = ao @ ow.T + ob
    return (ao + init_fv).astype(np.float32)
